# revision 32
# baseline (speedup 1.0000x reference)
"""Trainium2 Bass kernel for CausalWaveletFieldAttention (v4).

Sharding: (batch, head-half). Core c = (b = c//2, g = c%2) owns global
heads [8g, 8g+8) (512 channels) for the FULL 4096-token sequence, so the
causal wavelet conv needs no halo and no mid-kernel collectives. After
conv+skip, ONE AllToAll per 128-channel chunk exchanges field halves
(core keeps seq rows [2048g, 2048g+2048) of all 1024 channels); the
AllToAll slot index equals the source core parity, so the post-exchange
channel layout is identical on both cores and all tail weights are
uniform. Core c writes output rows [2048g, 2048g+2048) of batch b.

Per-core channel layout ("local"): position p in [0,512): chunk c=p//128,
local head h'=p%8 (global head 8g+h'), idx = 16c + (p%128)//8.
Post-exchange position P in [0,1024): slot j=P//512 (= source parity),
then local map with g=j.

Engine plan:
  - k-proj + gate-proj in fp8 DoubleRow (2 contraction chunks/matmul),
    v-proj fp16, out-proj bf16 (fp8 fails the error budget there).
  - kmag via block-ones matmul accumulated across chunks (psum held
    open), broadcast back with a [8,128] ones matmul.
  - conv: 13 large-shift taps as PE diagonal matmuls with range
    splitting (zero history is never materialized beyond a 128-col pad);
    11 small-shift taps as ScalarE scale-copies + DVE/GpSimd add chain;
    chain merged into the conv PSUM with an identity matmul.
  - skip taps on ScalarE (scale) + DVE (adds), pre-exchange, full seq.
  - coupling as 2 block-diag [128,128] matmuls per output chunk
    (contraction only over the 16 heads at equal idx).
  - gate kept in SBUF (no DRAM round trip), multiplied in by DVE.
"""

import ml_dtypes
import numpy as np

import concourse.bass as bass
import concourse.mybir as mybir
import concourse.tile as tile
from concourse import bacc
from concourse.bass_utils import run_bass_kernel_spmd

F32 = mybir.dt.float32
F16 = mybir.dt.float16
BF16 = mybir.dt.bfloat16
F8 = mybir.dt.float8e4
AF = mybir.ActivationFunctionType
DR = mybir.MatmulPerfMode.DoubleRow

NP_F8 = ml_dtypes.float8_e4m3fn
NP_BF = ml_dtypes.bfloat16

B, N, D, H, HD = 4, 4096, 1024, 16, 64
NCORES = 8
SEQ = N // 2            # rows per core in the tail phases
MYH = 8                 # heads per core
CH = 512                # channels per core
KC = CH // 128          # 4 local chunks
KK = D // 128           # 8 contraction chunks
PAD = 128               # zero pad in front of f0 for small-shift taps

D4 = [0.4829629131445341, 0.8365163037378079, 0.2241438680420134, -0.1294095225512604]
N_SCALES = 11
SHIFTS = [0, 1, 2, 3, 4, 6, 8, 12, 16, 24, 32, 48, 64, 96, 128, 192, 256,
          384, 512, 768, 1024, 1536, 2048, 3072]
PE_TAPS = [0, 24, 32, 48, 64, 96, 128, 192, 256, 384, 512, 768, 1024,
           1536, 2048, 3072]
SC_TAPS = [1, 3, 16]              # ScalarE scale-copy (1x mode, ~2.1us/2048)
DV_TAPS = [2, 4, 6, 8, 12]        # DVE TT-mul vs broadcast weights (2x mode)
GROUPS = [[0, 1], [2, 3], [4, 5], [6, 7]]

_PROGRAM_CACHE = {}


def _build_program(debug_outputs=False):
    key = bool(debug_outputs)
    if key in _PROGRAM_CACHE:
        return _PROGRAM_CACHE[key]

    nc = bacc.Bacc("TRN2", target_bir_lowering=False, debug=False,
                   num_devices=NCORES)

    # ---- DRAM parameters (per-core) ----
    x16T = nc.declare_dram_parameter("x16T", [128, KK, N], F16, isOutput=False)
    x8T = nc.declare_dram_parameter("x8T", [128, KK, N], F8, isOutput=False)
    xg8 = nc.declare_dram_parameter("xg8", [128, KK, SEQ], F8, isOutput=False)
    Wk8 = nc.declare_dram_parameter("Wk8", [128, KK, KC, 128], F8, isOutput=False)
    Wv16 = nc.declare_dram_parameter("Wv16", [128, KK, KC, 128], F16, isOutput=False)
    Wg8 = nc.declare_dram_parameter("Wg8", [128, KK, KK, 128], F8, isOutput=False)
    Wo16 = nc.declare_dram_parameter("Wo16", [128, KK, D], BF16, isOutput=False)
    Mdiag = nc.declare_dram_parameter("Mdiag", [128, len(PE_TAPS), 128], BF16,
                                      isOutput=False)
    Mcoup = nc.declare_dram_parameter("Mcoup", [128, 2, 2, 2, 128], BF16,
                                      isOutput=False)
    ident_in = nc.declare_dram_parameter("ident_in", [128, 128], BF16,
                                         isOutput=False)
    bo8_in = nc.declare_dram_parameter("bo8_in", [128, MYH], F16, isOutput=False)
    on8_in = nc.declare_dram_parameter("on8_in", [MYH, 128], BF16, isOutput=False)
    wtap_ew = nc.declare_dram_parameter("wtap_ew", [128, len(SC_TAPS)], F32,
                                        isOutput=False)
    Wdv = nc.declare_dram_parameter("Wdv", [128, len(DV_TAPS), 512], BF16,
                                    isOutput=False)
    ones1 = nc.declare_dram_parameter("ones1", [1, 128], mybir.dt.float32r,
                                      isOutput=False)
    bout1 = nc.declare_dram_parameter("bout1", [1, D], mybir.dt.float32r,
                                      isOutput=False)
    swt = nc.declare_dram_parameter("swt", [128, 2], F32, isOutput=False)
    bqkT = nc.declare_dram_parameter("bqkT", [128, KC], F32, isOutput=False)
    bvT = nc.declare_dram_parameter("bvT", [128, KC], F32, isOutput=False)
    bgT = nc.declare_dram_parameter("bgT", [128, KK], F32, isOutput=False)
    out = nc.declare_dram_parameter("out", [SEQ, D], F32, isOutput=True)

    dbg = {}
    if debug_outputs:
        for name, shape in (("dbg_f0", [128, KC, N]),
                            ("dbg_conv", [128, KC, N]),
                            ("dbg_field", [128, KC, N]),
                            ("dbg_km", [MYH, N]),
                            ("dbg_gate", [128, KK, SEQ]),
                            ("dbg_pgs", [128, KK, SEQ])):
            dbg[name] = nc.declare_dram_parameter(name, shape, BF16,
                                                  isOutput=True)

    # ---- internal DRAM for the exchange (pairwise AllGather per half) ----
    ag_in = [[nc.dram_tensor(f"ag_in{h}_{c}", [128, SEQ], BF16)
              for c in range(KC)] for h in range(2)]
    ag_out = [[nc.dram_tensor(f"ag_out{h}_{c}", [2, 128, SEQ], BF16)
               for c in range(KC)] for h in range(2)]

    with tile.TileContext(nc) as tc:
        with (
            tc.tile_pool(name="const", bufs=1) as constp,
            tc.tile_pool(name="p_long", bufs=1) as p_long,
        ):
            # ---- always-resident constants ----
            md_t = constp.tile([128, len(PE_TAPS), 128], BF16)
            nc.sync.dma_start(md_t[:], Mdiag[:])
            mc_t = constp.tile([128, 2, 2, 2, 128], BF16)
            nc.sync.dma_start(mc_t[:], Mcoup[:])
            id_t = constp.tile([128, 128], BF16)
            nc.sync.dma_start(id_t[:], ident_in[:])
            bo_t = constp.tile([128, MYH], F16)
            nc.sync.dma_start(bo_t[:], bo8_in[:])
            on_t = constp.tile([MYH, 128], BF16)
            nc.sync.dma_start(on_t[:], on8_in[:])
            wew_t = constp.tile([128, len(SC_TAPS)], F32)
            nc.sync.dma_start(wew_t[:], wtap_ew[:])
            wdv_t = constp.tile([128, len(DV_TAPS), 512], BF16)
            nc.sync.dma_start(wdv_t[:], Wdv[:])
            on1_t = constp.tile([1, 128], mybir.dt.float32r)
            nc.sync.dma_start(on1_t[:], ones1[:])
            bo1_t = constp.tile([1, D], mybir.dt.float32r)
            nc.sync.dma_start(bo1_t[:], bout1[:])
            swt_t = constp.tile([128, 2], F32)
            nc.sync.dma_start(swt_t[:], swt[:])
            bqk_t = constp.tile([128, KC], F32)
            nc.sync.dma_start(bqk_t[:], bqkT[:])
            bv_t = constp.tile([128, KC], F32)
            nc.sync.dma_start(bv_t[:], bvT[:])
            bg_t = constp.tile([128, KK], F32)
            nc.sync.dma_start(bg_t[:], bgT[:])

            km_sb = p_long.tile([MYH, N], BF16, tag="km_sb")
            kmb = p_long.tile([128, N], BF16, tag="kmb")

            with tc.tile_pool(name="px16", bufs=1) as px16:
                # allocate first, DMA after the K-phase inputs are queued
                x16 = px16.tile([128, KK, N], F16, tag="x16")
                wv_t = px16.tile([128, KK, KC, 128], F16, tag="wv_t")

                # ================= K phase: kmag =========================
                with (
                    tc.tile_pool(name="px8", bufs=1) as px8,
                    tc.tile_pool(name="pwk", bufs=2) as pwk,
                ):
                    x8 = px8.tile([128, KK, N], F8, tag="x8")
                    nc.sync.dma_start(x8[:], x8T[:])
                    wk_t = px8.tile([128, KK, KC, 128], F8, tag="wk_t")
                    nc.sync.dma_start(wk_t[:], Wk8[:])
                    # now the big v-phase loads (overlap with K compute)
                    nc.sync.dma_start(x16[:], x16T[:])
                    nc.sync.dma_start(wv_t[:], Wv16[:])

                    with (
                        tc.tile_pool(name="psk", bufs=1, space="PSUM") as psk,
                        tc.tile_pool(name="pskm", bufs=1, space="PSUM") as pskm,
                    ):
                        for half in range(2):
                            h0 = half * SEQ
                            kms = [pskm.tile([MYH, 512], F32, name=f"km{i}",
                                             tag=f"km{i}") for i in range(4)]
                            for c in range(KC):
                                kps = [psk.tile([128, 512], F32, name=f"kp{i}",
                                                tag=f"kp{i}") for i in range(4)]
                                for jp in range(KK // 2):
                                    for bk in range(4):
                                        nc.tensor.matmul(
                                            kps[bk][:],
                                            wk_t[:, 2 * jp:2 * jp + 2, c, :],
                                            x8[:, 2 * jp:2 * jp + 2,
                                               h0 + bk * 512:h0 + bk * 512 + 512],
                                            start=(jp == 0), stop=(jp == 3),
                                            perf_mode=DR)
                                k2t = pwk.tile([128, SEQ], F16, tag="k2t")
                                for bk in range(4):
                                    nc.scalar.activation(
                                        k2t[:, bk * 512:(bk + 1) * 512],
                                        kps[bk][:],
                                        AF.Square, bias=bqk_t[:, c:c + 1])
                                for bk in range(4):
                                    nc.tensor.matmul(
                                        kms[bk][:], bo_t[:],
                                        k2t[:, bk * 512:(bk + 1) * 512],
                                        start=(c == 0), stop=(c == KC - 1))
                            for bk in range(4):
                                nc.scalar.activation(
                                    km_sb[:, h0 + bk * 512:h0 + bk * 512 + 512],
                                    kms[bk][:], AF.Sqrt)

                    # kmag broadcast to all 128 partitions (same all chunks)
                    with tc.tile_pool(name="pskb", bufs=1,
                                      space="PSUM") as pskb:
                        kbs = [pskb.tile([128, 512], F32, name=f"kb{i}",
                                         tag=f"kb{i}") for i in range(4)]
                        for half in range(2):
                            h0 = half * SEQ
                            for bk in range(4):
                                nc.tensor.matmul(
                                    kbs[bk][:], on_t[:],
                                    km_sb[:, h0 + bk * 512:h0 + bk * 512 + 512],
                                    start=True, stop=True)
                            for bk in range(4):
                                nc.scalar.activation(
                                    kmb[:, h0 + bk * 512:h0 + bk * 512 + 512],
                                    kbs[bk][:], AF.Identity)
                if debug_outputs:
                    nc.sync.dma_start(dbg["dbg_km"][:], km_sb[:])

                # ============ V + conv pipeline ==========================
                with (
                    tc.tile_pool(name="pf0", bufs=3) as pf0,
                    tc.tile_pool(name="pwv", bufs=2) as pwv,
                    tc.tile_pool(name="pat", bufs=2) as pat,
                    tc.tile_pool(name="pacc", bufs=2) as pacc,
                    tc.tile_pool(name="pcb", bufs=3) as pcb,
                    tc.tile_pool(name="pcs", bufs=2) as pcs,
                    tc.tile_pool(name="pfl", bufs=1) as pfl,
                    tc.tile_pool(name="psv", bufs=1, space="PSUM") as psv,
                    tc.tile_pool(name="psc", bufs=1, space="PSUM") as psc,
                ):
                    f0ts = {}
                    cbts = {}

                    def v_work(c):
                        f0t = pf0.tile([128, PAD + N], BF16, tag="f0t")
                        f0ts[c] = f0t
                        if c < 3:
                            nc.gpsimd.memset(f0t[:, 0:PAD], 0.0)
                        for half in range(2):
                            h0 = half * SEQ
                            vps = [psv.tile([128, 512], F32, name=f"vp{i}",
                                            tag=f"vp{i}") for i in range(4)]
                            for kk in range(KK):
                                for bk in range(4):
                                    nc.tensor.matmul(
                                        vps[bk][:], wv_t[:, kk, c, :],
                                        x16[:, kk,
                                            h0 + bk * 512:h0 + bk * 512 + 512],
                                        start=(kk == 0), stop=(kk == KK - 1))
                            vbt = pwv.tile([128, SEQ], BF16, tag="vbt")
                            for bk in range(4):
                                nc.scalar.activation(
                                    vbt[:, bk * 512:(bk + 1) * 512], vps[bk][:],
                                    AF.Identity, bias=bv_t[:, c:c + 1])
                            nc.vector.tensor_mul(
                                f0t[:, PAD + h0:PAD + h0 + SEQ], vbt[:],
                                kmb[:, h0:h0 + SEQ])
                        if debug_outputs:
                            nc.sync.dma_start(dbg["dbg_f0"][:, c, :],
                                              f0t[:, PAD:])

                    def conv_ew(c):
                        """small-shift taps: DVE TT-mul (broadcast weights)
                        and ScalarE scale-copy, then DVE/GpSimd add chain."""
                        f0t = f0ts[c]
                        cbt = pcb.tile([128, N], BF16, tag="cbt")
                        cbts[c] = cbt
                        n_terms = len(DV_TAPS) + len(SC_TAPS)
                        for half in range(2):
                            base = PAD + half * SEQ
                            acc = None
                            nadd = 0
                            for i in range(n_terms):
                                att = pat.tile([128, SEQ], BF16, tag="att")
                                if i < len(DV_TAPS):
                                    s = DV_TAPS[i]
                                    for hb in range(4):
                                        o = base - s + hb * 512
                                        nc.vector.tensor_mul(
                                            att[:, hb * 512:(hb + 1) * 512],
                                            f0t[:, o:o + 512],
                                            wdv_t[:, i, :])
                                else:
                                    j = i - len(DV_TAPS)
                                    s = SC_TAPS[j]
                                    nc.scalar.activation(
                                        att[:], f0t[:, base - s:base - s + SEQ],
                                        AF.Identity, scale=wew_t[:, j:j + 1])
                                if acc is None:
                                    acc = att[:]
                                    continue
                                nadd += 1
                                if i == n_terms - 1:
                                    nxt = cbt[:, half * SEQ:(half + 1) * SEQ]
                                else:
                                    acct = pacc.tile([128, SEQ], BF16,
                                                     tag="acc", name="acct")
                                    nxt = acct[:]
                                if nadd == 3:
                                    nc.gpsimd.tensor_add(nxt, acc, att[:])
                                else:
                                    nc.vector.tensor_add(nxt, acc, att[:])
                                acc = nxt

                    def conv_pe(c):
                        """PE diag taps + chain merge + evict + skip + a2a."""
                        f0t = f0ts.pop(c)
                        cbt = cbts.pop(c)
                        cst = pcs.tile([128, N], BF16, tag="cst")
                        for chalf in range(2):
                            cps = [psc.tile([128, 512], F32, name=f"cp{i}",
                                            tag=f"cp{i}") for i in range(4)]
                            for ti, s in enumerate(PE_TAPS):
                                for bk in range(4):
                                    bg = chalf * 4 + bk
                                    lo = max(0, s - PAD - 512 * bg)
                                    if lo >= 512:
                                        continue
                                    src0 = PAD + 512 * bg + lo - s
                                    nc.tensor.matmul(
                                        cps[bk][:, lo:512], md_t[:, ti, :],
                                        f0t[:, src0:src0 + 512 - lo],
                                        start=(ti == 0), stop=False)
                            for bk in range(4):
                                bg = chalf * 4 + bk
                                nc.tensor.matmul(
                                    cps[bk][:], id_t[:],
                                    cbt[:, bg * 512:(bg + 1) * 512],
                                    start=False, stop=True)
                            for bk in range(4):
                                bg = chalf * 4 + bk
                                nc.scalar.activation(
                                    cst[:, bg * 512:(bg + 1) * 512],
                                    cps[bk][:], AF.Identity)
                        if debug_outputs:
                            nc.sync.dma_start(dbg["dbg_conv"][:, c, :], cst[:])
                        # skip taps: field = conv + sw0*conv[-512] + sw1*conv[-1024]
                        # temporaries borrow the pat/pacc rotating buffers
                        flt = pfl.tile([128, N], BF16, tag="flt")
                        t0a = pat.tile([128, SEQ], BF16, tag="att", name="t0a")
                        t1a = pat.tile([128, SEQ], BF16, tag="att", name="t1a")
                        nc.scalar.activation(t0a[:], cst[:, 0:SEQ],
                                             AF.Identity, scale=swt_t[:, 0:1])
                        nc.scalar.activation(t1a[:], cst[:, 0:SEQ],
                                             AF.Identity, scale=swt_t[:, 1:2])
                        ua = pacc.tile([128, 1024], BF16, tag="ua")
                        nc.vector.tensor_copy(flt[:, 0:512], cst[:, 0:512])
                        nc.vector.tensor_add(flt[:, 512:1024],
                                             cst[:, 512:1024], t0a[:, 0:512])
                        nc.vector.tensor_add(ua[:], t0a[:, 512:1536],
                                             t1a[:, 0:1024])
                        nc.vector.tensor_add(flt[:, 1024:2048],
                                             cst[:, 1024:2048], ua[:])
                        t0b = pat.tile([128, SEQ], BF16, tag="att", name="t0b")
                        t1b = pat.tile([128, SEQ], BF16, tag="att", name="t1b")
                        nc.scalar.activation(t0b[:], cst[:, 1536:1536 + SEQ],
                                             AF.Identity, scale=swt_t[:, 0:1])
                        nc.scalar.activation(t1b[:], cst[:, 1024:1024 + SEQ],
                                             AF.Identity, scale=swt_t[:, 1:2])
                        ub = pacc.tile([128, SEQ], BF16, tag="acc", name="ub")
                        nc.vector.tensor_add(ub[:], t0b[:], t1b[:])
                        nc.vector.tensor_add(flt[:, SEQ:], cst[:, SEQ:],
                                             ub[:])
                        if debug_outputs:
                            nc.sync.dma_start(dbg["dbg_field"][:, c, :], flt[:])
                        nc.sync.dma_start(ag_in[0][c][:], flt[:, 0:SEQ])
                        nc.sync.dma_start(ag_in[1][c][:], flt[:, SEQ:])
                        for h in range(2):
                            nc.gpsimd.collective_compute(
                                "AllGather", mybir.AluOpType.bypass,
                                replica_groups=GROUPS,
                                ins=[ag_in[h][c][:]], outs=[ag_out[h][c][:]])

                    for c in range(KC + 2):
                        if c < KC:
                            v_work(c)
                            conv_ew(c)
                        if c >= 2:
                            conv_pe(c - 2)

            # ================= tail: gate, couple, out ===================
            with (
                tc.tile_pool(name="ptail", bufs=1) as ptail,
                tc.tile_pool(name="pgt", bufs=2) as pgt,
                tc.tile_pool(name="pff", bufs=2) as pff,
                tc.tile_pool(name="pob", bufs=2) as pob,
            ):
                xg = ptail.tile([128, KK, SEQ], F8, tag="xg")
                nc.sync.dma_start(xg[:], xg8[:])
                wg_t = ptail.tile([128, KK, KK, 128], F8, tag="wg_t")
                nc.sync.dma_start(wg_t[:], Wg8[:])
                wo_t = ptail.tile([128, KK, D], BF16, tag="wo_t")
                nc.sync.dma_start(wo_t[:], Wo16[:])
                gate = ptail.tile([128, KK, SEQ], BF16, tag="gate")
                pgs = ptail.tile([128, KK, SEQ], BF16, tag="pgs")

                with tc.tile_pool(name="pscp", bufs=1, space="PSUM") as pscp:
                    with tc.tile_pool(name="psg", bufs=1, space="PSUM") as psg:
                        # gate = sigmoid(x @ (Wq@Wgate) + b'), fp8 DoubleRow
                        for q in range(KK):
                            gps = [psg.tile([128, 512], F32, name=f"gp{i}",
                                            tag=f"gp{i}") for i in range(4)]
                            for jp in range(KK // 2):
                                for bk in range(4):
                                    nc.tensor.matmul(
                                        gps[bk][:],
                                        wg_t[:, 2 * jp:2 * jp + 2, q, :],
                                        xg[:, 2 * jp:2 * jp + 2,
                                           bk * 512:bk * 512 + 512],
                                        start=(jp == 0), stop=(jp == 3),
                                        perf_mode=DR)
                            for bk in range(4):
                                nc.scalar.activation(
                                    gate[:, q, bk * 512:(bk + 1) * 512],
                                    gps[bk][:],
                                    AF.Sigmoid, bias=bg_t[:, q:q + 1])
                        if debug_outputs:
                            nc.sync.dma_start(dbg["dbg_gate"][:], gate[:])

                        # coupling + gate multiply, per exchange chunk.
                        # mc_t[:, jo, hh, ji, :] is host-zeroed unless hh
                        # equals this core's parity, selecting which gather
                        # half (L/R) feeds the couple without runtime offsets.
                        for c in range(KC):
                            ffh = []
                            for h in range(2):
                                fft = pff.tile([128, 2, SEQ], BF16, tag="fft",
                                               name=f"fft{h}")
                                nc.sync.dma_start(
                                    fft[:],
                                    ag_out[h][c][:].rearrange("j p n -> p j n"))
                                ffh.append(fft)
                            for jo in range(2):
                                q = jo * KC + c
                                cpp = [pscp.tile([128, 512], F32,
                                                 name=f"cc{i}", tag=f"cc{i}")
                                       for i in range(4)]
                                for bk in range(4):
                                    first = True
                                    for hh in range(2):
                                        for ji in range(2):
                                            nc.tensor.matmul(
                                                cpp[bk][:],
                                                mc_t[:, jo, hh, ji, :],
                                                ffh[hh][:, ji,
                                                        bk * 512:(bk + 1) * 512],
                                                start=first,
                                                stop=(hh == 1 and ji == 1))
                                            first = False
                                for bk in range(4):
                                    nc.vector.tensor_mul(
                                        pgs[:, q, bk * 512:(bk + 1) * 512],
                                        cpp[bk][:],
                                        gate[:, q, bk * 512:(bk + 1) * 512])
                        if debug_outputs:
                            nc.sync.dma_start(dbg["dbg_pgs"][:], pgs[:])

                    # out = pgs.T @ Wout + bout (bias via rank-1 f32r matmul,
                    # result DMAed PSUM -> DRAM directly)
                    with tc.tile_pool(name="pso", bufs=2, space="PSUM") as pso:
                        for st in range(SEQ // 128):
                            ops = [pso.tile([128, 512], F32, name=f"op{i}",
                                            tag=f"op{i}") for i in range(2)]
                            for cb in range(2):
                                nc.tensor.matmul(
                                    ops[cb][:], on1_t[:],
                                    bo1_t[:, cb * 512:(cb + 1) * 512],
                                    start=True, stop=False)
                            for q in range(KK):
                                for cb in range(2):
                                    nc.tensor.matmul(
                                        ops[cb][:],
                                        pgs[:, q, st * 128:(st + 1) * 128],
                                        wo_t[:, q, cb * 512:(cb + 1) * 512],
                                        start=False, stop=(q == KK - 1))
                            outb = pob.tile([128, D], F32, tag="outb")
                            for cb in range(2):
                                nc.vector.tensor_copy(
                                    outb[:, cb * 512:(cb + 1) * 512],
                                    ops[cb][:])
                            nc.sync.dma_start(out[st * 128:(st + 1) * 128, :],
                                              outb[:])

    nc.compile()
    _PROGRAM_CACHE[key] = nc
    return nc


def _softmax(a, axis):
    a = a - a.max(axis=axis, keepdims=True)
    e = np.exp(a)
    return e / e.sum(axis=axis, keepdims=True)


def _locmap(g):
    """local position p (0..511) -> original channel index (0..1023)."""
    p = np.arange(CH)
    return (MYH * g + p % MYH) * HD + 16 * (p // 128) + (p % 128) // MYH


def _host_prep(inputs):
    x = np.asarray(inputs["x"], np.float32)
    Wqkv = np.asarray(inputs["Wqkv"], np.float64)
    bqkv = np.asarray(inputs["bqkv"], np.float64)
    Wout = np.asarray(inputs["Wout"], np.float64)
    bout = np.asarray(inputs["bout"], np.float32)
    Wgate = np.asarray(inputs["Wgate"], np.float64)
    bgate = np.asarray(inputs["bgate"], np.float64)
    scale_gain = np.asarray(inputs["scale_gain"], np.float64)
    skip_w = np.asarray(inputs["skip_w"], np.float64)
    coupling = np.asarray(inputs["coupling"], np.float64)

    gains = _softmax(scale_gain, axis=0)              # [11, H]
    sw = 1.0 / (1.0 + np.exp(-skip_w))                # [2]
    coup = _softmax(coupling, axis=-1)                # [H, H]

    sidx = {s: i for i, s in enumerate(SHIFTS)}
    wtab = np.zeros((len(SHIFTS), H), np.float64)
    for j in range(N_SCALES):
        d = 1 << j
        for t in range(4):
            wtab[sidx[(3 - t) * d]] += D4[t] * gains[j]

    Wq = Wqkv[:, :D]
    Wk = Wqkv[:, D:2 * D]
    Wv = Wqkv[:, 2 * D:]
    Wqg = Wq @ Wgate                                  # folded gate proj
    bg_full = bqkv[:D] @ Wgate + bgate

    postmap = np.concatenate([_locmap(0), _locmap(1)])  # [1024]

    # uniform (parity-independent) tensors
    r = np.arange(128)
    Wg8 = np.zeros((128, KK, KK, 128), NP_F8)
    Wo16 = np.zeros((128, KK, D), NP_BF)
    bgT = np.zeros((128, KK), np.float32)
    for q in range(KK):
        cols = postmap[q * 128 + np.arange(128)]
        for kk in range(KK):
            Wg8[:, kk, q, :] = Wqg[kk * 128:(kk + 1) * 128, cols].astype(NP_F8)
        Wo16[:, q, :] = Wout[cols, :].astype(NP_BF)
        bgT[:, q] = bg_full[cols].astype(np.float32)

    rr = np.arange(128)[:, None]
    mm = np.arange(128)[None, :]
    same_idx = (rr // MYH) == (mm // MYH)
    coup_blk = np.zeros((2, 2, 128, 128), np.float64)
    for jo in range(2):
        for ji in range(2):
            coup_blk[jo, ji] = (coup[MYH * jo + mm % MYH, MYH * ji + rr % MYH]
                                * same_idx)

    bo8 = np.zeros((128, MYH), np.float16)
    bo8[r, r % MYH] = 1.0
    on8 = np.zeros((MYH, 128), NP_BF)
    on8[r % MYH, r] = 1.0
    ident = np.eye(128, dtype=NP_BF)
    swt = np.broadcast_to(sw.astype(np.float32), (128, 2)).copy()
    ones1 = np.ones((1, 128), np.float32)
    bout1 = bout.reshape(1, D).astype(np.float32)

    shared = dict(Wg8=Wg8, Wo16=Wo16, bgT=bgT, bo8_in=bo8,
                  on8_in=on8, ident_in=ident, swt=swt, ones1=ones1,
                  bout1=bout1)

    in_maps = []
    for core in range(NCORES):
        b, g = core // 2, core % 2
        loc = _locmap(g)

        # coupling stationary: gather-half hh contributes only when hh == g
        Mc = np.zeros((128, 2, 2, 2, 128), NP_BF)
        for jo in range(2):
            for ji in range(2):
                Mc[:, jo, g, ji, :] = coup_blk[jo, ji].astype(NP_BF)
        heads = MYH * g + np.arange(128) % MYH        # head of partition r

        Wk8 = np.zeros((128, KK, KC, 128), NP_F8)
        Wv16 = np.zeros((128, KK, KC, 128), np.float16)
        for c in range(KC):
            cols = loc[c * 128 + np.arange(128)]
            for kk in range(KK):
                Wk8[:, kk, c, :] = Wk[kk * 128:(kk + 1) * 128, cols].astype(NP_F8)
                Wv16[:, kk, c, :] = Wv[kk * 128:(kk + 1) * 128, cols].astype(np.float16)
        bqkT = bqkv[D + loc].reshape(KC, 128).T.copy().astype(np.float32)
        bvT = bqkv[2 * D + loc].reshape(KC, 128).T.copy().astype(np.float32)

        Md = np.zeros((128, len(PE_TAPS), 128), NP_BF)
        for ti, s in enumerate(PE_TAPS):
            Md[r, ti, r] = wtab[sidx[s], heads].astype(NP_BF)
        wew = np.zeros((128, len(SC_TAPS)), np.float32)
        for i, s in enumerate(SC_TAPS):
            wew[:, i] = wtab[sidx[s], heads].astype(np.float32)
        wdv = np.zeros((128, len(DV_TAPS), 512), NP_BF)
        for i, s in enumerate(DV_TAPS):
            wdv[:, i, :] = np.broadcast_to(
                wtab[sidx[s], heads].astype(NP_BF)[:, None], (128, 512))

        xb = x[b]                                     # [N, D]
        x16T = np.ascontiguousarray(
            xb.T.reshape(KK, 128, N).transpose(1, 0, 2)).astype(np.float16)
        x8T = x16T.astype(NP_F8)
        xg = xb[g * SEQ:(g + 1) * SEQ, :]             # [SEQ, D]
        xg8 = np.ascontiguousarray(
            xg.T.reshape(KK, 128, SEQ).transpose(1, 0, 2)).astype(NP_F8)

        in_maps.append(dict(x16T=x16T, x8T=x8T, xg8=xg8, Wk8=Wk8, Wv16=Wv16,
                            Mdiag=Md, wtap_ew=wew, Wdv=wdv, bqkT=bqkT,
                            bvT=bvT, Mcoup=Mc, **shared))
    return in_maps


def run_cores(inputs, debug_outputs=False, trace=False):
    nc = _build_program(debug_outputs=debug_outputs)
    in_maps = _host_prep(inputs)
    res = run_bass_kernel_spmd(nc, in_maps, list(range(NCORES)), trace=trace)
    return res


def kernel(**inputs) -> np.ndarray:
    res = run_cores(inputs)
    out = np.empty((B, N, D), np.float32)
    for c in range(NCORES):
        b, g = c // 2, c % 2
        out[b, g * SEQ:(g + 1) * SEQ, :] = res.results[c]["out"]
    return out


# revision 33
# speedup vs baseline: 1.2145x; 1.2145x over previous
"""Trainium2 Bass kernel for CausalWaveletFieldAttention (v4).

Sharding: (batch, head-half). Core c = (b = c//2, g = c%2) owns global
heads [8g, 8g+8) (512 channels) for the FULL 4096-token sequence, so the
causal wavelet conv needs no halo and no mid-kernel collectives. After
conv+skip, ONE AllToAll per 128-channel chunk exchanges field halves
(core keeps seq rows [2048g, 2048g+2048) of all 1024 channels); the
AllToAll slot index equals the source core parity, so the post-exchange
channel layout is identical on both cores and all tail weights are
uniform. Core c writes output rows [2048g, 2048g+2048) of batch b.

Per-core channel layout ("local"): position p in [0,512): chunk c=p//128,
local head h'=p%8 (global head 8g+h'), idx = 16c + (p%128)//8.
Post-exchange position P in [0,1024): slot j=P//512 (= source parity),
then local map with g=j.

Engine plan:
  - k-proj + gate-proj in fp8 DoubleRow (2 contraction chunks/matmul),
    v-proj fp16, out-proj bf16 (fp8 fails the error budget there).
  - kmag via block-ones matmul accumulated across chunks (psum held
    open), broadcast back with a [8,128] ones matmul.
  - conv: 13 large-shift taps as PE diagonal matmuls with range
    splitting (zero history is never materialized beyond a 128-col pad);
    11 small-shift taps as ScalarE scale-copies + DVE/GpSimd add chain;
    chain merged into the conv PSUM with an identity matmul.
  - skip taps on ScalarE (scale) + DVE (adds), pre-exchange, full seq.
  - coupling as 2 block-diag [128,128] matmuls per output chunk
    (contraction only over the 16 heads at equal idx).
  - gate kept in SBUF (no DRAM round trip), multiplied in by DVE.
"""

import ml_dtypes
import numpy as np

import concourse.bass as bass
import concourse.mybir as mybir
import concourse.tile as tile
from concourse import bacc
from concourse.bass_utils import run_bass_kernel_spmd

F32 = mybir.dt.float32
F16 = mybir.dt.float16
BF16 = mybir.dt.bfloat16
F8 = mybir.dt.float8e4
AF = mybir.ActivationFunctionType
DR = mybir.MatmulPerfMode.DoubleRow

NP_F8 = ml_dtypes.float8_e4m3fn
NP_BF = ml_dtypes.bfloat16

B, N, D, H, HD = 4, 4096, 1024, 16, 64
NCORES = 8
SEQ = N // 2            # rows per core in the tail phases
MYH = 8                 # heads per core
CH = 512                # channels per core
KC = CH // 128          # 4 local chunks
KK = D // 128           # 8 contraction chunks
PAD = 128               # zero pad in front of f0 for small-shift taps

D4 = [0.4829629131445341, 0.8365163037378079, 0.2241438680420134, -0.1294095225512604]
N_SCALES = 11
SHIFTS = [0, 1, 2, 3, 4, 6, 8, 12, 16, 24, 32, 48, 64, 96, 128, 192, 256,
          384, 512, 768, 1024, 1536, 2048, 3072]
PE_TAPS = [0, 24, 32, 48, 64, 96, 128, 192, 256, 384, 512, 768, 1024,
           1536, 2048, 3072]
SC_TAPS = [1, 3, 16]              # ScalarE scale-copy (1x mode, ~2.1us/2048)
DV_TAPS = [2, 4, 6, 8, 12]        # DVE TT-mul vs broadcast weights (2x mode)
GROUPS = [[0, 1], [2, 3], [4, 5], [6, 7]]

_PROGRAM_CACHE = {}


def _build_program(debug_outputs=False):
    key = bool(debug_outputs)
    if key in _PROGRAM_CACHE:
        return _PROGRAM_CACHE[key]

    nc = bacc.Bacc("TRN2", target_bir_lowering=False, debug=False,
                   num_devices=NCORES)

    # ---- DRAM parameters (per-core) ----
    x16T = nc.declare_dram_parameter("x16T", [128, KK, N], F16, isOutput=False)
    x8T = nc.declare_dram_parameter("x8T", [128, KK, N], F8, isOutput=False)
    xg8 = nc.declare_dram_parameter("xg8", [128, KK, SEQ], F8, isOutput=False)
    Wk8 = nc.declare_dram_parameter("Wk8", [128, KK, KC, 128], F8, isOutput=False)
    Wv16 = nc.declare_dram_parameter("Wv16", [128, KK, KC, 128], F16, isOutput=False)
    Wg8 = nc.declare_dram_parameter("Wg8", [128, KK, KK, 128], F8, isOutput=False)
    Wo16 = nc.declare_dram_parameter("Wo16", [128, KK, D], BF16, isOutput=False)
    Mdiag = nc.declare_dram_parameter("Mdiag", [128, len(PE_TAPS), 128], BF16,
                                      isOutput=False)
    Mcoup = nc.declare_dram_parameter("Mcoup", [128, 2, 2, 2, 128], BF16,
                                      isOutput=False)
    ident_in = nc.declare_dram_parameter("ident_in", [128, 128], BF16,
                                         isOutput=False)
    bo8_in = nc.declare_dram_parameter("bo8_in", [128, MYH], F16, isOutput=False)
    on8_in = nc.declare_dram_parameter("on8_in", [MYH, 128], BF16, isOutput=False)
    wtap_ew = nc.declare_dram_parameter("wtap_ew", [128, len(SC_TAPS)], F32,
                                        isOutput=False)
    Wdv = nc.declare_dram_parameter("Wdv", [128, len(DV_TAPS), 1024], BF16,
                                    isOutput=False)
    ones1 = nc.declare_dram_parameter("ones1", [1, 128], mybir.dt.float32r,
                                      isOutput=False)
    bout1 = nc.declare_dram_parameter("bout1", [1, D], mybir.dt.float32r,
                                      isOutput=False)
    swt = nc.declare_dram_parameter("swt", [128, 2], F32, isOutput=False)
    bqkT = nc.declare_dram_parameter("bqkT", [128, KC], F32, isOutput=False)
    bvT = nc.declare_dram_parameter("bvT", [128, KC], F32, isOutput=False)
    bgT = nc.declare_dram_parameter("bgT", [128, KK], F32, isOutput=False)
    out = nc.declare_dram_parameter("out", [SEQ, D], F32, isOutput=True)

    dbg = {}
    if debug_outputs:
        for name, shape in (("dbg_f0", [128, KC, N]),
                            ("dbg_conv", [128, KC, N]),
                            ("dbg_field", [128, KC, N]),
                            ("dbg_km", [MYH, N]),
                            ("dbg_gate", [128, KK, SEQ]),
                            ("dbg_pgs", [128, KK, SEQ])):
            dbg[name] = nc.declare_dram_parameter(name, shape, BF16,
                                                  isOutput=True)

    # ---- internal DRAM for the exchange (pairwise AllGather per half) ----
    ag_in = [[nc.dram_tensor(f"ag_in{h}_{c}", [128, SEQ], BF16)
              for c in range(KC)] for h in range(2)]
    ag_out = [[nc.dram_tensor(f"ag_out{h}_{c}", [2, 128, SEQ], BF16)
               for c in range(KC)] for h in range(2)]

    with tile.TileContext(nc) as tc:
        with (
            tc.tile_pool(name="const", bufs=1) as constp,
            tc.tile_pool(name="p_long", bufs=1) as p_long,
        ):
            # ---- always-resident constants ----
            md_t = constp.tile([128, len(PE_TAPS), 128], BF16)
            nc.sync.dma_start(md_t[:], Mdiag[:])
            mc_t = constp.tile([128, 2, 2, 2, 128], BF16)
            nc.sync.dma_start(mc_t[:], Mcoup[:])
            id_t = constp.tile([128, 128], BF16)
            nc.sync.dma_start(id_t[:], ident_in[:])
            bo_t = constp.tile([128, MYH], F16)
            nc.sync.dma_start(bo_t[:], bo8_in[:])
            on_t = constp.tile([MYH, 128], BF16)
            nc.sync.dma_start(on_t[:], on8_in[:])
            wew_t = constp.tile([128, len(SC_TAPS)], F32)
            nc.sync.dma_start(wew_t[:], wtap_ew[:])
            wdv_t = constp.tile([128, len(DV_TAPS), 1024], BF16)
            nc.sync.dma_start(wdv_t[:], Wdv[:])
            on1_t = constp.tile([1, 128], mybir.dt.float32r)
            nc.sync.dma_start(on1_t[:], ones1[:])
            bo1_t = constp.tile([1, D], mybir.dt.float32r)
            nc.sync.dma_start(bo1_t[:], bout1[:])
            swt_t = constp.tile([128, 2], F32)
            nc.sync.dma_start(swt_t[:], swt[:])
            bqk_t = constp.tile([128, KC], F32)
            nc.sync.dma_start(bqk_t[:], bqkT[:])
            bv_t = constp.tile([128, KC], F32)
            nc.sync.dma_start(bv_t[:], bvT[:])
            bg_t = constp.tile([128, KK], F32)
            nc.sync.dma_start(bg_t[:], bgT[:])

            km_sb = p_long.tile([MYH, N], BF16, tag="km_sb")
            kmb = p_long.tile([128, N], BF16, tag="kmb")

            with tc.tile_pool(name="px16", bufs=1) as px16:
                # allocate first, DMA after the K-phase inputs are queued
                x16 = px16.tile([128, KK, N], F16, tag="x16")
                wv_t = px16.tile([128, KK, KC, 128], F16, tag="wv_t")

                # ================= K phase: kmag =========================
                with (
                    tc.tile_pool(name="px8", bufs=1) as px8,
                    tc.tile_pool(name="pwk", bufs=2) as pwk,
                ):
                    x8 = px8.tile([128, KK, N], F8, tag="x8")
                    nc.sync.dma_start(x8[:], x8T[:])
                    wk_t = px8.tile([128, KK, KC, 128], F8, tag="wk_t")
                    nc.sync.dma_start(wk_t[:], Wk8[:])
                    # now the big v-phase loads (overlap with K compute)
                    nc.sync.dma_start(x16[:], x16T[:])
                    nc.sync.dma_start(wv_t[:], Wv16[:])

                    with (
                        tc.tile_pool(name="psk", bufs=1, space="PSUM") as psk,
                        tc.tile_pool(name="pskm", bufs=1, space="PSUM") as pskm,
                    ):
                        for half in range(2):
                            h0 = half * SEQ
                            kms = [pskm.tile([MYH, 512], F32, name=f"km{i}",
                                             tag=f"km{i}") for i in range(4)]
                            for c in range(KC):
                                kps = [psk.tile([128, 512], F32, name=f"kp{i}",
                                                tag=f"kp{i}") for i in range(4)]
                                for jp in range(KK // 2):
                                    for bk in range(4):
                                        nc.tensor.matmul(
                                            kps[bk][:],
                                            wk_t[:, 2 * jp:2 * jp + 2, c, :],
                                            x8[:, 2 * jp:2 * jp + 2,
                                               h0 + bk * 512:h0 + bk * 512 + 512],
                                            start=(jp == 0), stop=(jp == 3),
                                            perf_mode=DR)
                                k2t = pwk.tile([128, SEQ], F16, tag="k2t")
                                for bk in range(4):
                                    nc.scalar.activation(
                                        k2t[:, bk * 512:(bk + 1) * 512],
                                        kps[bk][:],
                                        AF.Square, bias=bqk_t[:, c:c + 1])
                                for bk in range(4):
                                    nc.tensor.matmul(
                                        kms[bk][:], bo_t[:],
                                        k2t[:, bk * 512:(bk + 1) * 512],
                                        start=(c == 0), stop=(c == KC - 1))
                            for bk in range(4):
                                nc.scalar.activation(
                                    km_sb[:, h0 + bk * 512:h0 + bk * 512 + 512],
                                    kms[bk][:], AF.Sqrt)

                    # kmag broadcast to all 128 partitions (same all chunks)
                    with tc.tile_pool(name="pskb", bufs=1,
                                      space="PSUM") as pskb:
                        kbs = [pskb.tile([128, 512], F32, name=f"kb{i}",
                                         tag=f"kb{i}") for i in range(4)]
                        for half in range(2):
                            h0 = half * SEQ
                            for bk in range(4):
                                nc.tensor.matmul(
                                    kbs[bk][:], on_t[:],
                                    km_sb[:, h0 + bk * 512:h0 + bk * 512 + 512],
                                    start=True, stop=True)
                            for bk in range(4):
                                nc.scalar.activation(
                                    kmb[:, h0 + bk * 512:h0 + bk * 512 + 512],
                                    kbs[bk][:], AF.Identity)
                if debug_outputs:
                    nc.sync.dma_start(dbg["dbg_km"][:], km_sb[:])

                # ============ V + conv pipeline ==========================
                with (
                    tc.tile_pool(name="pf0", bufs=2) as pf0,
                    tc.tile_pool(name="pwv", bufs=2) as pwv,
                    tc.tile_pool(name="pat", bufs=3) as pat,
                    tc.tile_pool(name="pacc", bufs=2) as pacc,
                    tc.tile_pool(name="pcb", bufs=2) as pcb,
                    tc.tile_pool(name="pcs", bufs=2) as pcs,
                    tc.tile_pool(name="pfl", bufs=1) as pfl,
                    tc.tile_pool(name="psv", bufs=1, space="PSUM") as psv,
                    tc.tile_pool(name="psc", bufs=1, space="PSUM") as psc,
                ):
                    f0ts = {}
                    cbts = {}

                    def v_work(c):
                        f0t = pf0.tile([128, PAD + N], BF16, tag="f0t")
                        f0ts[c] = f0t
                        if c < 2:
                            nc.gpsimd.memset(f0t[:, 0:PAD], 0.0)
                        for half in range(2):
                            h0 = half * SEQ
                            vps = [psv.tile([128, 512], F32, name=f"vp{i}",
                                            tag=f"vp{i}") for i in range(4)]
                            for kk in range(KK):
                                for bk in range(4):
                                    nc.tensor.matmul(
                                        vps[bk][:], wv_t[:, kk, c, :],
                                        x16[:, kk,
                                            h0 + bk * 512:h0 + bk * 512 + 512],
                                        start=(kk == 0), stop=(kk == KK - 1))
                            vbt = pwv.tile([128, SEQ], BF16, tag="vbt")
                            for bk in range(4):
                                nc.scalar.activation(
                                    vbt[:, bk * 512:(bk + 1) * 512], vps[bk][:],
                                    AF.Identity, bias=bv_t[:, c:c + 1])
                            nc.vector.tensor_mul(
                                f0t[:, PAD + h0:PAD + h0 + SEQ], vbt[:],
                                kmb[:, h0:h0 + SEQ])
                        if debug_outputs:
                            nc.sync.dma_start(dbg["dbg_f0"][:, c, :],
                                              f0t[:, PAD:])

                    def conv_ew(c):
                        """small-shift taps: DVE TT-mul (broadcast weights)
                        and ScalarE scale-copy, then DVE/GpSimd add chain."""
                        f0t = f0ts[c]
                        cbt = pcb.tile([128, N], BF16, tag="cbt")
                        cbts[c] = cbt
                        n_terms = len(DV_TAPS) + len(SC_TAPS)
                        for half in range(2):
                            base = PAD + half * SEQ
                            acc = None
                            nadd = 0
                            for i in range(n_terms):
                                att = pat.tile([128, SEQ], BF16, tag="att")
                                if i < len(DV_TAPS):
                                    s = DV_TAPS[i]
                                    for hb in range(2):
                                        o = base - s + hb * 1024
                                        nc.vector.tensor_mul(
                                            att[:, hb * 1024:(hb + 1) * 1024],
                                            f0t[:, o:o + 1024],
                                            wdv_t[:, i, :])
                                else:
                                    j = i - len(DV_TAPS)
                                    s = SC_TAPS[j]
                                    nc.scalar.activation(
                                        att[:], f0t[:, base - s:base - s + SEQ],
                                        AF.Identity, scale=wew_t[:, j:j + 1])
                                if acc is None:
                                    acc = att[:]
                                    continue
                                nadd += 1
                                if i == n_terms - 1:
                                    nxt = cbt[:, half * SEQ:(half + 1) * SEQ]
                                else:
                                    acct = pacc.tile([128, SEQ], BF16,
                                                     tag="acc", name="acct")
                                    nxt = acct[:]
                                if nadd == 3:
                                    nc.gpsimd.tensor_add(nxt, acc, att[:])
                                else:
                                    nc.vector.tensor_add(nxt, acc, att[:])
                                acc = nxt

                    def conv_pe(c):
                        """PE diag taps + chain merge + evict + skip + a2a."""
                        f0t = f0ts.pop(c)
                        cbt = cbts.pop(c)
                        cst = pcs.tile([128, N], BF16, tag="cst")
                        for chalf in range(2):
                            cps = [psc.tile([128, 512], F32, name=f"cp{i}",
                                            tag=f"cp{i}") for i in range(4)]
                            for ti, s in enumerate(PE_TAPS):
                                for bk in range(4):
                                    bg = chalf * 4 + bk
                                    lo = max(0, s - PAD - 512 * bg)
                                    if lo >= 512:
                                        continue
                                    src0 = PAD + 512 * bg + lo - s
                                    nc.tensor.matmul(
                                        cps[bk][:, lo:512], md_t[:, ti, :],
                                        f0t[:, src0:src0 + 512 - lo],
                                        start=(ti == 0), stop=False)
                            for bk in range(4):
                                bg = chalf * 4 + bk
                                nc.tensor.matmul(
                                    cps[bk][:], id_t[:],
                                    cbt[:, bg * 512:(bg + 1) * 512],
                                    start=False, stop=True)
                            for bk in range(4):
                                bg = chalf * 4 + bk
                                nc.scalar.activation(
                                    cst[:, bg * 512:(bg + 1) * 512],
                                    cps[bk][:], AF.Identity)
                        if debug_outputs:
                            nc.sync.dma_start(dbg["dbg_conv"][:, c, :], cst[:])
                        # skip taps: field = conv + sw0*conv[-512] + sw1*conv[-1024]
                        # temporaries borrow the pat/pacc rotating buffers
                        flt = pfl.tile([128, N], BF16, tag="flt")
                        t0a = pat.tile([128, SEQ], BF16, tag="att", name="t0a")
                        t1a = pat.tile([128, SEQ], BF16, tag="att", name="t1a")
                        nc.scalar.activation(t0a[:], cst[:, 0:SEQ],
                                             AF.Identity, scale=swt_t[:, 0:1])
                        nc.scalar.activation(t1a[:], cst[:, 0:SEQ],
                                             AF.Identity, scale=swt_t[:, 1:2])
                        ua = pacc.tile([128, 1024], BF16, tag="ua")
                        nc.vector.tensor_copy(flt[:, 0:512], cst[:, 0:512])
                        nc.vector.tensor_add(flt[:, 512:1024],
                                             cst[:, 512:1024], t0a[:, 0:512])
                        nc.vector.tensor_add(ua[:], t0a[:, 512:1536],
                                             t1a[:, 0:1024])
                        nc.vector.tensor_add(flt[:, 1024:2048],
                                             cst[:, 1024:2048], ua[:])
                        t0b = pat.tile([128, SEQ], BF16, tag="att", name="t0b")
                        t1b = pat.tile([128, SEQ], BF16, tag="att", name="t1b")
                        nc.scalar.activation(t0b[:], cst[:, 1536:1536 + SEQ],
                                             AF.Identity, scale=swt_t[:, 0:1])
                        nc.scalar.activation(t1b[:], cst[:, 1024:1024 + SEQ],
                                             AF.Identity, scale=swt_t[:, 1:2])
                        ub = pacc.tile([128, SEQ], BF16, tag="acc", name="ub")
                        nc.vector.tensor_add(ub[:], t0b[:], t1b[:])
                        nc.vector.tensor_add(flt[:, SEQ:], cst[:, SEQ:],
                                             ub[:])
                        if debug_outputs:
                            nc.sync.dma_start(dbg["dbg_field"][:, c, :], flt[:])
                        nc.sync.dma_start(ag_in[0][c][:], flt[:, 0:SEQ])
                        nc.sync.dma_start(ag_in[1][c][:], flt[:, SEQ:])
                        for h in range(2):
                            nc.gpsimd.collective_compute(
                                "AllGather", mybir.AluOpType.bypass,
                                replica_groups=GROUPS,
                                ins=[ag_in[h][c][:]], outs=[ag_out[h][c][:]])

                    for c in range(KC + 1):
                        if c < KC:
                            v_work(c)
                            conv_ew(c)
                        if c >= 1:
                            conv_pe(c - 1)

            # ================= tail: gate, couple, out ===================
            with (
                tc.tile_pool(name="ptail", bufs=1) as ptail,
                tc.tile_pool(name="pgt", bufs=2) as pgt,
                tc.tile_pool(name="pff", bufs=2) as pff,
                tc.tile_pool(name="pob", bufs=2) as pob,
            ):
                xg = ptail.tile([128, KK, SEQ], F8, tag="xg")
                nc.sync.dma_start(xg[:], xg8[:])
                wg_t = ptail.tile([128, KK, KK, 128], F8, tag="wg_t")
                nc.sync.dma_start(wg_t[:], Wg8[:])
                wo_t = ptail.tile([128, KK, D], BF16, tag="wo_t")
                nc.sync.dma_start(wo_t[:], Wo16[:])
                gate = ptail.tile([128, KK, SEQ], BF16, tag="gate")
                pgs = ptail.tile([128, KK, SEQ], BF16, tag="pgs")

                with tc.tile_pool(name="pscp", bufs=1, space="PSUM") as pscp:
                    with tc.tile_pool(name="psg", bufs=1, space="PSUM") as psg:
                        # gate = sigmoid(x @ (Wq@Wgate) + b'), fp8 DoubleRow
                        for q in range(KK):
                            gps = [psg.tile([128, 512], F32, name=f"gp{i}",
                                            tag=f"gp{i}") for i in range(4)]
                            for jp in range(KK // 2):
                                for bk in range(4):
                                    nc.tensor.matmul(
                                        gps[bk][:],
                                        wg_t[:, 2 * jp:2 * jp + 2, q, :],
                                        xg[:, 2 * jp:2 * jp + 2,
                                           bk * 512:bk * 512 + 512],
                                        start=(jp == 0), stop=(jp == 3),
                                        perf_mode=DR)
                            for bk in range(4):
                                nc.scalar.activation(
                                    gate[:, q, bk * 512:(bk + 1) * 512],
                                    gps[bk][:],
                                    AF.Sigmoid, bias=bg_t[:, q:q + 1])
                        if debug_outputs:
                            nc.sync.dma_start(dbg["dbg_gate"][:], gate[:])

                        # coupling + gate multiply, per exchange chunk.
                        # mc_t[:, jo, hh, ji, :] is host-zeroed unless hh
                        # equals this core's parity, selecting which gather
                        # half (L/R) feeds the couple without runtime offsets.
                        for c in range(KC):
                            ffh = []
                            for h in range(2):
                                fft = pff.tile([128, 2, SEQ], BF16, tag="fft",
                                               name=f"fft{h}")
                                nc.sync.dma_start(
                                    fft[:],
                                    ag_out[h][c][:].rearrange("j p n -> p j n"))
                                ffh.append(fft)
                            for jo in range(2):
                                q = jo * KC + c
                                cpp = [pscp.tile([128, 512], F32,
                                                 name=f"cc{i}", tag=f"cc{i}")
                                       for i in range(4)]
                                for bk in range(4):
                                    first = True
                                    for hh in range(2):
                                        for ji in range(2):
                                            nc.tensor.matmul(
                                                cpp[bk][:],
                                                mc_t[:, jo, hh, ji, :],
                                                ffh[hh][:, ji,
                                                        bk * 512:(bk + 1) * 512],
                                                start=first,
                                                stop=(hh == 1 and ji == 1))
                                            first = False
                                for bk in range(4):
                                    nc.vector.tensor_mul(
                                        pgs[:, q, bk * 512:(bk + 1) * 512],
                                        cpp[bk][:],
                                        gate[:, q, bk * 512:(bk + 1) * 512])
                        if debug_outputs:
                            nc.sync.dma_start(dbg["dbg_pgs"][:], pgs[:])

                    # out = pgs.T @ Wout + bout (bias via rank-1 f32r matmul,
                    # result DMAed PSUM -> DRAM directly)
                    with tc.tile_pool(name="pso", bufs=2, space="PSUM") as pso:
                        for st in range(SEQ // 128):
                            ops = [pso.tile([128, 512], F32, name=f"op{i}",
                                            tag=f"op{i}") for i in range(2)]
                            for cb in range(2):
                                nc.tensor.matmul(
                                    ops[cb][:], on1_t[:],
                                    bo1_t[:, cb * 512:(cb + 1) * 512],
                                    start=True, stop=False)
                            for q in range(KK):
                                for cb in range(2):
                                    nc.tensor.matmul(
                                        ops[cb][:],
                                        pgs[:, q, st * 128:(st + 1) * 128],
                                        wo_t[:, q, cb * 512:(cb + 1) * 512],
                                        start=False, stop=(q == KK - 1))
                            outb = pob.tile([128, D], F32, tag="outb")
                            for cb in range(2):
                                nc.vector.tensor_copy(
                                    outb[:, cb * 512:(cb + 1) * 512],
                                    ops[cb][:])
                            nc.sync.dma_start(out[st * 128:(st + 1) * 128, :],
                                              outb[:])

    nc.compile()
    _PROGRAM_CACHE[key] = nc
    return nc


def _softmax(a, axis):
    a = a - a.max(axis=axis, keepdims=True)
    e = np.exp(a)
    return e / e.sum(axis=axis, keepdims=True)


def _locmap(g):
    """local position p (0..511) -> original channel index (0..1023)."""
    p = np.arange(CH)
    return (MYH * g + p % MYH) * HD + 16 * (p // 128) + (p % 128) // MYH


def _host_prep(inputs):
    x = np.asarray(inputs["x"], np.float32)
    Wqkv = np.asarray(inputs["Wqkv"], np.float64)
    bqkv = np.asarray(inputs["bqkv"], np.float64)
    Wout = np.asarray(inputs["Wout"], np.float64)
    bout = np.asarray(inputs["bout"], np.float32)
    Wgate = np.asarray(inputs["Wgate"], np.float64)
    bgate = np.asarray(inputs["bgate"], np.float64)
    scale_gain = np.asarray(inputs["scale_gain"], np.float64)
    skip_w = np.asarray(inputs["skip_w"], np.float64)
    coupling = np.asarray(inputs["coupling"], np.float64)

    gains = _softmax(scale_gain, axis=0)              # [11, H]
    sw = 1.0 / (1.0 + np.exp(-skip_w))                # [2]
    coup = _softmax(coupling, axis=-1)                # [H, H]

    sidx = {s: i for i, s in enumerate(SHIFTS)}
    wtab = np.zeros((len(SHIFTS), H), np.float64)
    for j in range(N_SCALES):
        d = 1 << j
        for t in range(4):
            wtab[sidx[(3 - t) * d]] += D4[t] * gains[j]

    Wq = Wqkv[:, :D]
    Wk = Wqkv[:, D:2 * D]
    Wv = Wqkv[:, 2 * D:]
    Wqg = Wq @ Wgate                                  # folded gate proj
    bg_full = bqkv[:D] @ Wgate + bgate

    postmap = np.concatenate([_locmap(0), _locmap(1)])  # [1024]

    # uniform (parity-independent) tensors
    r = np.arange(128)
    Wg8 = np.zeros((128, KK, KK, 128), NP_F8)
    Wo16 = np.zeros((128, KK, D), NP_BF)
    bgT = np.zeros((128, KK), np.float32)
    for q in range(KK):
        cols = postmap[q * 128 + np.arange(128)]
        for kk in range(KK):
            Wg8[:, kk, q, :] = Wqg[kk * 128:(kk + 1) * 128, cols].astype(NP_F8)
        Wo16[:, q, :] = Wout[cols, :].astype(NP_BF)
        bgT[:, q] = bg_full[cols].astype(np.float32)

    rr = np.arange(128)[:, None]
    mm = np.arange(128)[None, :]
    same_idx = (rr // MYH) == (mm // MYH)
    coup_blk = np.zeros((2, 2, 128, 128), np.float64)
    for jo in range(2):
        for ji in range(2):
            coup_blk[jo, ji] = (coup[MYH * jo + mm % MYH, MYH * ji + rr % MYH]
                                * same_idx)

    bo8 = np.zeros((128, MYH), np.float16)
    bo8[r, r % MYH] = 1.0
    on8 = np.zeros((MYH, 128), NP_BF)
    on8[r % MYH, r] = 1.0
    ident = np.eye(128, dtype=NP_BF)
    swt = np.broadcast_to(sw.astype(np.float32), (128, 2)).copy()
    ones1 = np.ones((1, 128), np.float32)
    bout1 = bout.reshape(1, D).astype(np.float32)

    shared = dict(Wg8=Wg8, Wo16=Wo16, bgT=bgT, bo8_in=bo8,
                  on8_in=on8, ident_in=ident, swt=swt, ones1=ones1,
                  bout1=bout1)

    in_maps = []
    for core in range(NCORES):
        b, g = core // 2, core % 2
        loc = _locmap(g)

        # coupling stationary: gather-half hh contributes only when hh == g
        Mc = np.zeros((128, 2, 2, 2, 128), NP_BF)
        for jo in range(2):
            for ji in range(2):
                Mc[:, jo, g, ji, :] = coup_blk[jo, ji].astype(NP_BF)
        heads = MYH * g + np.arange(128) % MYH        # head of partition r

        Wk8 = np.zeros((128, KK, KC, 128), NP_F8)
        Wv16 = np.zeros((128, KK, KC, 128), np.float16)
        for c in range(KC):
            cols = loc[c * 128 + np.arange(128)]
            for kk in range(KK):
                Wk8[:, kk, c, :] = Wk[kk * 128:(kk + 1) * 128, cols].astype(NP_F8)
                Wv16[:, kk, c, :] = Wv[kk * 128:(kk + 1) * 128, cols].astype(np.float16)
        bqkT = bqkv[D + loc].reshape(KC, 128).T.copy().astype(np.float32)
        bvT = bqkv[2 * D + loc].reshape(KC, 128).T.copy().astype(np.float32)

        Md = np.zeros((128, len(PE_TAPS), 128), NP_BF)
        for ti, s in enumerate(PE_TAPS):
            Md[r, ti, r] = wtab[sidx[s], heads].astype(NP_BF)
        wew = np.zeros((128, len(SC_TAPS)), np.float32)
        for i, s in enumerate(SC_TAPS):
            wew[:, i] = wtab[sidx[s], heads].astype(np.float32)
        wdv = np.zeros((128, len(DV_TAPS), 1024), NP_BF)
        for i, s in enumerate(DV_TAPS):
            wdv[:, i, :] = np.broadcast_to(
                wtab[sidx[s], heads].astype(NP_BF)[:, None], (128, 1024))

        xb = x[b]                                     # [N, D]
        x16T = np.ascontiguousarray(
            xb.T.reshape(KK, 128, N).transpose(1, 0, 2)).astype(np.float16)
        x8T = x16T.astype(NP_F8)
        xg = xb[g * SEQ:(g + 1) * SEQ, :]             # [SEQ, D]
        xg8 = np.ascontiguousarray(
            xg.T.reshape(KK, 128, SEQ).transpose(1, 0, 2)).astype(NP_F8)

        in_maps.append(dict(x16T=x16T, x8T=x8T, xg8=xg8, Wk8=Wk8, Wv16=Wv16,
                            Mdiag=Md, wtap_ew=wew, Wdv=wdv, bqkT=bqkT,
                            bvT=bvT, Mcoup=Mc, **shared))
    return in_maps


def run_cores(inputs, debug_outputs=False, trace=False):
    nc = _build_program(debug_outputs=debug_outputs)
    in_maps = _host_prep(inputs)
    res = run_bass_kernel_spmd(nc, in_maps, list(range(NCORES)), trace=trace)
    return res


def kernel(**inputs) -> np.ndarray:
    res = run_cores(inputs)
    out = np.empty((B, N, D), np.float32)
    for c in range(NCORES):
        b, g = c // 2, c % 2
        out[b, g * SEQ:(g + 1) * SEQ, :] = res.results[c]["out"]
    return out


# revision 34
# speedup vs baseline: 1.2448x; 1.0250x over previous
"""Trainium2 Bass kernel for CausalWaveletFieldAttention (v4).

Sharding: (batch, head-half). Core c = (b = c//2, g = c%2) owns global
heads [8g, 8g+8) (512 channels) for the FULL 4096-token sequence, so the
causal wavelet conv needs no halo and no mid-kernel collectives. After
conv+skip, ONE AllToAll per 128-channel chunk exchanges field halves
(core keeps seq rows [2048g, 2048g+2048) of all 1024 channels); the
AllToAll slot index equals the source core parity, so the post-exchange
channel layout is identical on both cores and all tail weights are
uniform. Core c writes output rows [2048g, 2048g+2048) of batch b.

Per-core channel layout ("local"): position p in [0,512): chunk c=p//128,
local head h'=p%8 (global head 8g+h'), idx = 16c + (p%128)//8.
Post-exchange position P in [0,1024): slot j=P//512 (= source parity),
then local map with g=j.

Engine plan:
  - k-proj + gate-proj in fp8 DoubleRow (2 contraction chunks/matmul),
    v-proj fp16, out-proj bf16 (fp8 fails the error budget there).
  - kmag via block-ones matmul accumulated across chunks (psum held
    open), broadcast back with a [8,128] ones matmul.
  - conv: 13 large-shift taps as PE diagonal matmuls with range
    splitting (zero history is never materialized beyond a 128-col pad);
    11 small-shift taps as ScalarE scale-copies + DVE/GpSimd add chain;
    chain merged into the conv PSUM with an identity matmul.
  - skip taps on ScalarE (scale) + DVE (adds), pre-exchange, full seq.
  - coupling as 2 block-diag [128,128] matmuls per output chunk
    (contraction only over the 16 heads at equal idx).
  - gate kept in SBUF (no DRAM round trip), multiplied in by DVE.
"""

import ml_dtypes
import numpy as np

import concourse.bass as bass
import concourse.mybir as mybir
import concourse.tile as tile
from concourse import bacc
from concourse.bass_utils import run_bass_kernel_spmd

F32 = mybir.dt.float32
F16 = mybir.dt.float16
BF16 = mybir.dt.bfloat16
F8 = mybir.dt.float8e4
AF = mybir.ActivationFunctionType
DR = mybir.MatmulPerfMode.DoubleRow

NP_F8 = ml_dtypes.float8_e4m3fn
NP_BF = ml_dtypes.bfloat16

B, N, D, H, HD = 4, 4096, 1024, 16, 64
NCORES = 8
SEQ = N // 2            # rows per core in the tail phases
MYH = 8                 # heads per core
CH = 512                # channels per core
KC = CH // 128          # 4 local chunks
KK = D // 128           # 8 contraction chunks
PAD = 128               # zero pad in front of f0 for small-shift taps

D4 = [0.4829629131445341, 0.8365163037378079, 0.2241438680420134, -0.1294095225512604]
N_SCALES = 11
SHIFTS = [0, 1, 2, 3, 4, 6, 8, 12, 16, 24, 32, 48, 64, 96, 128, 192, 256,
          384, 512, 768, 1024, 1536, 2048, 3072]
PE_TAPS = [0, 24, 32, 48, 64, 96, 128, 192, 256, 384, 512, 768, 1024,
           1536, 2048, 3072]
SC_TAPS = [1, 3, 16]              # ScalarE scale-copy (1x mode, ~2.1us/2048)
DV_TAPS = [2, 4, 6, 8, 12]        # DVE TT-mul vs broadcast weights (2x mode)
GROUPS = [[0, 1], [2, 3], [4, 5], [6, 7]]

_PROGRAM_CACHE = {}


def _build_program(debug_outputs=False):
    key = bool(debug_outputs)
    if key in _PROGRAM_CACHE:
        return _PROGRAM_CACHE[key]

    nc = bacc.Bacc("TRN2", target_bir_lowering=False, debug=False,
                   num_devices=NCORES)

    # ---- DRAM parameters (per-core) ----
    x16T = nc.declare_dram_parameter("x16T", [128, KK, N], F16, isOutput=False)
    x8T = nc.declare_dram_parameter("x8T", [128, KK, N], F8, isOutput=False)
    xg8 = nc.declare_dram_parameter("xg8", [128, KK, SEQ], F8, isOutput=False)
    Wk8 = nc.declare_dram_parameter("Wk8", [128, KK, KC, 128], F8, isOutput=False)
    Wv16 = nc.declare_dram_parameter("Wv16", [128, KK, KC, 128], F16, isOutput=False)
    Wg8 = nc.declare_dram_parameter("Wg8", [128, KK, KK, 128], F8, isOutput=False)
    Wo16 = nc.declare_dram_parameter("Wo16", [128, KK, D], BF16, isOutput=False)
    Mdiag = nc.declare_dram_parameter("Mdiag", [128, len(PE_TAPS), 128], BF16,
                                      isOutput=False)
    Mcoup = nc.declare_dram_parameter("Mcoup", [128, 2, 2, 2, 128], BF16,
                                      isOutput=False)
    ident_in = nc.declare_dram_parameter("ident_in", [128, 128], BF16,
                                         isOutput=False)
    bo8_in = nc.declare_dram_parameter("bo8_in", [128, MYH], F16, isOutput=False)
    on8_in = nc.declare_dram_parameter("on8_in", [MYH, 128], BF16, isOutput=False)
    wtap_ew = nc.declare_dram_parameter("wtap_ew", [128, len(SC_TAPS)], F32,
                                        isOutput=False)
    Wdv = nc.declare_dram_parameter("Wdv", [128, len(DV_TAPS), 1024], BF16,
                                    isOutput=False)
    ones1 = nc.declare_dram_parameter("ones1", [1, 128], mybir.dt.float32r,
                                      isOutput=False)
    bout1 = nc.declare_dram_parameter("bout1", [1, D], mybir.dt.float32r,
                                      isOutput=False)
    swt = nc.declare_dram_parameter("swt", [128, 2], F32, isOutput=False)
    bqkT = nc.declare_dram_parameter("bqkT", [128, KC], F32, isOutput=False)
    bvT = nc.declare_dram_parameter("bvT", [128, KC], F32, isOutput=False)
    bgT = nc.declare_dram_parameter("bgT", [128, KK], F32, isOutput=False)
    out = nc.declare_dram_parameter("out", [SEQ, D], F32, isOutput=True)

    dbg = {}
    if debug_outputs:
        for name, shape in (("dbg_f0", [128, KC, N]),
                            ("dbg_conv", [128, KC, N]),
                            ("dbg_field", [128, KC, N]),
                            ("dbg_km", [MYH, N]),
                            ("dbg_gate", [128, KK, SEQ]),
                            ("dbg_pgs", [128, KK, SEQ])):
            dbg[name] = nc.declare_dram_parameter(name, shape, BF16,
                                                  isOutput=True)

    # ---- internal DRAM for the exchange (one pairwise AllGather/chunk) ----
    ag_in = [nc.dram_tensor(f"ag_in{c}", [128, N], BF16) for c in range(KC)]
    ag_out = [nc.dram_tensor(f"ag_out{c}", [2, 128, N], BF16)
              for c in range(KC)]

    with tile.TileContext(nc) as tc:
        with (
            tc.tile_pool(name="const", bufs=1) as constp,
            tc.tile_pool(name="p_long", bufs=1) as p_long,
        ):
            # ---- always-resident constants ----
            md_t = constp.tile([128, len(PE_TAPS), 128], BF16)
            nc.sync.dma_start(md_t[:], Mdiag[:])
            mc_t = constp.tile([128, 2, 2, 2, 128], BF16)
            nc.sync.dma_start(mc_t[:], Mcoup[:])
            id_t = constp.tile([128, 128], BF16)
            nc.sync.dma_start(id_t[:], ident_in[:])
            bo_t = constp.tile([128, MYH], F16)
            nc.sync.dma_start(bo_t[:], bo8_in[:])
            on_t = constp.tile([MYH, 128], BF16)
            nc.sync.dma_start(on_t[:], on8_in[:])
            wew_t = constp.tile([128, len(SC_TAPS)], F32)
            nc.sync.dma_start(wew_t[:], wtap_ew[:])
            wdv_t = constp.tile([128, len(DV_TAPS), 1024], BF16)
            nc.sync.dma_start(wdv_t[:], Wdv[:])
            on1_t = constp.tile([1, 128], mybir.dt.float32r)
            nc.sync.dma_start(on1_t[:], ones1[:])
            bo1_t = constp.tile([1, D], mybir.dt.float32r)
            nc.sync.dma_start(bo1_t[:], bout1[:])
            swt_t = constp.tile([128, 2], F32)
            nc.sync.dma_start(swt_t[:], swt[:])
            bqk_t = constp.tile([128, KC], F32)
            nc.sync.dma_start(bqk_t[:], bqkT[:])
            bv_t = constp.tile([128, KC], F32)
            nc.sync.dma_start(bv_t[:], bvT[:])
            bg_t = constp.tile([128, KK], F32)
            nc.sync.dma_start(bg_t[:], bgT[:])

            km_sb = p_long.tile([MYH, N], BF16, tag="km_sb")
            kmb = p_long.tile([128, N], BF16, tag="kmb")

            with tc.tile_pool(name="px16", bufs=1) as px16:
                # allocate first, DMA after the K-phase inputs are queued
                x16 = px16.tile([128, KK, N], F16, tag="x16")
                wv_t = px16.tile([128, KK, KC, 128], F16, tag="wv_t")

                # ================= K phase: kmag =========================
                with (
                    tc.tile_pool(name="px8", bufs=1) as px8,
                    tc.tile_pool(name="pwk", bufs=2) as pwk,
                ):
                    x8 = px8.tile([128, KK, N], F8, tag="x8")
                    wk_t = px8.tile([128, KK, KC, 128], F8, tag="wk_t")
                    nc.sync.dma_start(wk_t[:], Wk8[:])
                    for half in range(2):
                        h0 = half * SEQ
                        for jp in range(KK // 2):
                            nc.sync.dma_start(
                                x8[:, 2 * jp:2 * jp + 2, h0:h0 + SEQ],
                                x8T[:, 2 * jp:2 * jp + 2, h0:h0 + SEQ])
                    # now the big v-phase loads (overlap with K compute)
                    nc.sync.dma_start(x16[:], x16T[:])
                    nc.sync.dma_start(wv_t[:], Wv16[:])

                    with (
                        tc.tile_pool(name="psk", bufs=1, space="PSUM") as psk,
                        tc.tile_pool(name="pskm", bufs=1, space="PSUM") as pskm,
                    ):
                        for half in range(2):
                            h0 = half * SEQ
                            kms = [pskm.tile([MYH, 512], F32, name=f"km{i}",
                                             tag=f"km{i}") for i in range(4)]
                            for c in range(KC):
                                kps = [psk.tile([128, 512], F32, name=f"kp{i}",
                                                tag=f"kp{i}") for i in range(4)]
                                for jp in range(KK // 2):
                                    for bk in range(4):
                                        nc.tensor.matmul(
                                            kps[bk][:],
                                            wk_t[:, 2 * jp:2 * jp + 2, c, :],
                                            x8[:, 2 * jp:2 * jp + 2,
                                               h0 + bk * 512:h0 + bk * 512 + 512],
                                            start=(jp == 0), stop=(jp == 3),
                                            perf_mode=DR)
                                k2t = pwk.tile([128, SEQ], F16, tag="k2t")
                                for bk in range(4):
                                    nc.scalar.activation(
                                        k2t[:, bk * 512:(bk + 1) * 512],
                                        kps[bk][:],
                                        AF.Square, bias=bqk_t[:, c:c + 1])
                                for bk in range(4):
                                    nc.tensor.matmul(
                                        kms[bk][:], bo_t[:],
                                        k2t[:, bk * 512:(bk + 1) * 512],
                                        start=(c == 0), stop=(c == KC - 1))
                            for bk in range(4):
                                nc.scalar.activation(
                                    km_sb[:, h0 + bk * 512:h0 + bk * 512 + 512],
                                    kms[bk][:], AF.Sqrt)

                    # kmag broadcast to all 128 partitions (same all chunks)
                    with tc.tile_pool(name="pskb", bufs=1,
                                      space="PSUM") as pskb:
                        kbs = [pskb.tile([128, 512], F32, name=f"kb{i}",
                                         tag=f"kb{i}") for i in range(4)]
                        for half in range(2):
                            h0 = half * SEQ
                            for bk in range(4):
                                nc.tensor.matmul(
                                    kbs[bk][:], on_t[:],
                                    km_sb[:, h0 + bk * 512:h0 + bk * 512 + 512],
                                    start=True, stop=True)
                            for bk in range(4):
                                nc.scalar.activation(
                                    kmb[:, h0 + bk * 512:h0 + bk * 512 + 512],
                                    kbs[bk][:], AF.Identity)
                if debug_outputs:
                    nc.sync.dma_start(dbg["dbg_km"][:], km_sb[:])

                # ============ V + conv pipeline ==========================
                with (
                    tc.tile_pool(name="pf0", bufs=2) as pf0,
                    tc.tile_pool(name="pwv", bufs=2) as pwv,
                    tc.tile_pool(name="pat", bufs=3) as pat,
                    tc.tile_pool(name="pacc", bufs=2) as pacc,
                    tc.tile_pool(name="pcb", bufs=2) as pcb,
                    tc.tile_pool(name="pcs", bufs=2) as pcs,
                    tc.tile_pool(name="pfl", bufs=1) as pfl,
                    tc.tile_pool(name="psv", bufs=1, space="PSUM") as psv,
                    tc.tile_pool(name="psc", bufs=1, space="PSUM") as psc,
                ):
                    f0ts = {}
                    cbts = {}

                    def v_work(c):
                        f0t = pf0.tile([128, PAD + N], BF16, tag="f0t")
                        f0ts[c] = f0t
                        if c < 2:
                            nc.gpsimd.memset(f0t[:, 0:PAD], 0.0)
                        for half in range(2):
                            h0 = half * SEQ
                            vps = [psv.tile([128, 512], F32, name=f"vp{i}",
                                            tag=f"vp{i}") for i in range(4)]
                            for kk in range(KK):
                                for bk in range(4):
                                    nc.tensor.matmul(
                                        vps[bk][:], wv_t[:, kk, c, :],
                                        x16[:, kk,
                                            h0 + bk * 512:h0 + bk * 512 + 512],
                                        start=(kk == 0), stop=(kk == KK - 1))
                            vbt = pwv.tile([128, SEQ], BF16, tag="vbt")
                            for bk in range(4):
                                nc.scalar.activation(
                                    vbt[:, bk * 512:(bk + 1) * 512], vps[bk][:],
                                    AF.Identity, bias=bv_t[:, c:c + 1])
                            nc.vector.tensor_mul(
                                f0t[:, PAD + h0:PAD + h0 + SEQ], vbt[:],
                                kmb[:, h0:h0 + SEQ])
                        if debug_outputs:
                            nc.sync.dma_start(dbg["dbg_f0"][:, c, :],
                                              f0t[:, PAD:])

                    def conv_ew(c):
                        """small-shift taps: DVE TT-mul (broadcast weights)
                        and ScalarE scale-copy, then DVE/GpSimd add chain."""
                        f0t = f0ts[c]
                        cbt = pcb.tile([128, N], BF16, tag="cbt")
                        cbts[c] = cbt
                        n_terms = len(DV_TAPS) + len(SC_TAPS)
                        for half in range(2):
                            base = PAD + half * SEQ
                            acc = None
                            nadd = 0
                            for i in range(n_terms):
                                att = pat.tile([128, SEQ], BF16, tag="att")
                                if i < len(DV_TAPS):
                                    s = DV_TAPS[i]
                                    for hb in range(2):
                                        o = base - s + hb * 1024
                                        nc.vector.tensor_mul(
                                            att[:, hb * 1024:(hb + 1) * 1024],
                                            f0t[:, o:o + 1024],
                                            wdv_t[:, i, :])
                                else:
                                    j = i - len(DV_TAPS)
                                    s = SC_TAPS[j]
                                    nc.scalar.activation(
                                        att[:], f0t[:, base - s:base - s + SEQ],
                                        AF.Identity, scale=wew_t[:, j:j + 1])
                                if acc is None:
                                    acc = att[:]
                                    continue
                                nadd += 1
                                if i == n_terms - 1:
                                    nxt = cbt[:, half * SEQ:(half + 1) * SEQ]
                                else:
                                    acct = pacc.tile([128, SEQ], BF16,
                                                     tag="acc", name="acct")
                                    nxt = acct[:]
                                if nadd == 3:
                                    nc.gpsimd.tensor_add(nxt, acc, att[:])
                                else:
                                    nc.vector.tensor_add(nxt, acc, att[:])
                                acc = nxt

                    def conv_pe(c):
                        """PE diag taps + chain merge + evict + skip + a2a."""
                        f0t = f0ts.pop(c)
                        cbt = cbts.pop(c)
                        cst = pcs.tile([128, N], BF16, tag="cst")
                        for chalf in range(2):
                            cps = [psc.tile([128, 512], F32, name=f"cp{i}",
                                            tag=f"cp{i}") for i in range(4)]
                            for ti, s in enumerate(PE_TAPS):
                                for bk in range(4):
                                    bg = chalf * 4 + bk
                                    lo = max(0, s - PAD - 512 * bg)
                                    if lo >= 512:
                                        continue
                                    src0 = PAD + 512 * bg + lo - s
                                    nc.tensor.matmul(
                                        cps[bk][:, lo:512], md_t[:, ti, :],
                                        f0t[:, src0:src0 + 512 - lo],
                                        start=(ti == 0), stop=False)
                            for bk in range(4):
                                bg = chalf * 4 + bk
                                nc.tensor.matmul(
                                    cps[bk][:], id_t[:],
                                    cbt[:, bg * 512:(bg + 1) * 512],
                                    start=False, stop=True)
                            for bk in range(4):
                                bg = chalf * 4 + bk
                                nc.scalar.activation(
                                    cst[:, bg * 512:(bg + 1) * 512],
                                    cps[bk][:], AF.Identity)
                        if debug_outputs:
                            nc.sync.dma_start(dbg["dbg_conv"][:, c, :], cst[:])
                        # skip taps: field = conv + sw0*conv[-512] + sw1*conv[-1024]
                        # temporaries borrow the pat/pacc rotating buffers
                        flt = pfl.tile([128, N], BF16, tag="flt")
                        t0a = pat.tile([128, SEQ], BF16, tag="att", name="t0a")
                        t1a = pat.tile([128, SEQ], BF16, tag="att", name="t1a")
                        nc.scalar.activation(t0a[:], cst[:, 0:SEQ],
                                             AF.Identity, scale=swt_t[:, 0:1])
                        nc.scalar.activation(t1a[:], cst[:, 0:SEQ],
                                             AF.Identity, scale=swt_t[:, 1:2])
                        ua = pacc.tile([128, 1024], BF16, tag="ua")
                        nc.vector.tensor_copy(flt[:, 0:512], cst[:, 0:512])
                        nc.vector.tensor_add(flt[:, 512:1024],
                                             cst[:, 512:1024], t0a[:, 0:512])
                        nc.vector.tensor_add(ua[:], t0a[:, 512:1536],
                                             t1a[:, 0:1024])
                        nc.vector.tensor_add(flt[:, 1024:2048],
                                             cst[:, 1024:2048], ua[:])
                        t0b = pat.tile([128, SEQ], BF16, tag="att", name="t0b")
                        t1b = pat.tile([128, SEQ], BF16, tag="att", name="t1b")
                        nc.scalar.activation(t0b[:], cst[:, 1536:1536 + SEQ],
                                             AF.Identity, scale=swt_t[:, 0:1])
                        nc.scalar.activation(t1b[:], cst[:, 1024:1024 + SEQ],
                                             AF.Identity, scale=swt_t[:, 1:2])
                        ub = pacc.tile([128, SEQ], BF16, tag="acc", name="ub")
                        nc.vector.tensor_add(ub[:], t0b[:], t1b[:])
                        nc.vector.tensor_add(flt[:, SEQ:], cst[:, SEQ:],
                                             ub[:])
                        if debug_outputs:
                            nc.sync.dma_start(dbg["dbg_field"][:, c, :], flt[:])
                        nc.sync.dma_start(ag_in[c][:], flt[:])
                        nc.gpsimd.collective_compute(
                            "AllGather", mybir.AluOpType.bypass,
                            replica_groups=GROUPS,
                            ins=[ag_in[c][:]], outs=[ag_out[c][:]])

                    for c in range(KC + 1):
                        if c < KC:
                            v_work(c)
                            conv_ew(c)
                        if c >= 1:
                            conv_pe(c - 1)

            # ================= tail: gate, couple, out ===================
            with (
                tc.tile_pool(name="ptail", bufs=1) as ptail,
                tc.tile_pool(name="pgt", bufs=2) as pgt,
                tc.tile_pool(name="pff", bufs=2) as pff,
                tc.tile_pool(name="pob", bufs=2) as pob,
            ):
                xg = ptail.tile([128, KK, SEQ], F8, tag="xg")
                nc.sync.dma_start(xg[:], xg8[:])
                wg_t = ptail.tile([128, KK, KK, 128], F8, tag="wg_t")
                nc.sync.dma_start(wg_t[:], Wg8[:])
                wo_t = ptail.tile([128, KK, D], BF16, tag="wo_t")
                nc.sync.dma_start(wo_t[:], Wo16[:])
                gate = ptail.tile([128, KK, SEQ], BF16, tag="gate")
                pgs = ptail.tile([128, KK, SEQ], BF16, tag="pgs")

                with tc.tile_pool(name="pscp", bufs=1, space="PSUM") as pscp:
                    with tc.tile_pool(name="psg", bufs=1, space="PSUM") as psg:
                        # gate = sigmoid(x @ (Wq@Wgate) + b'), fp8 DoubleRow
                        for q in range(KK):
                            gps = [psg.tile([128, 512], F32, name=f"gp{i}",
                                            tag=f"gp{i}") for i in range(4)]
                            for jp in range(KK // 2):
                                for bk in range(4):
                                    nc.tensor.matmul(
                                        gps[bk][:],
                                        wg_t[:, 2 * jp:2 * jp + 2, q, :],
                                        xg[:, 2 * jp:2 * jp + 2,
                                           bk * 512:bk * 512 + 512],
                                        start=(jp == 0), stop=(jp == 3),
                                        perf_mode=DR)
                            for bk in range(4):
                                nc.scalar.activation(
                                    gate[:, q, bk * 512:(bk + 1) * 512],
                                    gps[bk][:],
                                    AF.Sigmoid, bias=bg_t[:, q:q + 1])
                        if debug_outputs:
                            nc.sync.dma_start(dbg["dbg_gate"][:], gate[:])

                        # coupling + gate multiply, per exchange chunk.
                        # mc_t[:, jo, hh, ji, :] is host-zeroed unless hh
                        # equals this core's parity, selecting which gather
                        # half (L/R) feeds the couple without runtime offsets.
                        for c in range(KC):
                            fft = pff.tile([128, 2, N], BF16, tag="fft")
                            nc.sync.dma_start(
                                fft[:],
                                ag_out[c][:].rearrange("j p n -> p j n"))
                            for jo in range(2):
                                q = jo * KC + c
                                cpp = [pscp.tile([128, 512], F32,
                                                 name=f"cc{i}", tag=f"cc{i}")
                                       for i in range(4)]
                                for bk in range(4):
                                    first = True
                                    for hh in range(2):
                                        for ji in range(2):
                                            nc.tensor.matmul(
                                                cpp[bk][:],
                                                mc_t[:, jo, hh, ji, :],
                                                fft[:, ji, hh * SEQ + bk * 512:
                                                    hh * SEQ + (bk + 1) * 512],
                                                start=first,
                                                stop=(hh == 1 and ji == 1))
                                            first = False
                                for bk in range(4):
                                    nc.vector.tensor_mul(
                                        pgs[:, q, bk * 512:(bk + 1) * 512],
                                        cpp[bk][:],
                                        gate[:, q, bk * 512:(bk + 1) * 512])
                        if debug_outputs:
                            nc.sync.dma_start(dbg["dbg_pgs"][:], pgs[:])

                    # out = pgs.T @ Wout + bout (bias via rank-1 f32r matmul,
                    # result DMAed PSUM -> DRAM directly)
                    with tc.tile_pool(name="pso", bufs=2, space="PSUM") as pso:
                        for st in range(SEQ // 128):
                            ops = [pso.tile([128, 512], F32, name=f"op{i}",
                                            tag=f"op{i}") for i in range(2)]
                            for cb in range(2):
                                nc.tensor.matmul(
                                    ops[cb][:], on1_t[:],
                                    bo1_t[:, cb * 512:(cb + 1) * 512],
                                    start=True, stop=False)
                            for q in range(KK):
                                for cb in range(2):
                                    nc.tensor.matmul(
                                        ops[cb][:],
                                        pgs[:, q, st * 128:(st + 1) * 128],
                                        wo_t[:, q, cb * 512:(cb + 1) * 512],
                                        start=False, stop=(q == KK - 1))
                            outb = pob.tile([128, D], F32, tag="outb")
                            for cb in range(2):
                                nc.vector.tensor_copy(
                                    outb[:, cb * 512:(cb + 1) * 512],
                                    ops[cb][:])
                            nc.sync.dma_start(out[st * 128:(st + 1) * 128, :],
                                              outb[:])

    nc.compile()
    _PROGRAM_CACHE[key] = nc
    return nc


def _softmax(a, axis):
    a = a - a.max(axis=axis, keepdims=True)
    e = np.exp(a)
    return e / e.sum(axis=axis, keepdims=True)


def _locmap(g):
    """local position p (0..511) -> original channel index (0..1023)."""
    p = np.arange(CH)
    return (MYH * g + p % MYH) * HD + 16 * (p // 128) + (p % 128) // MYH


def _host_prep(inputs):
    x = np.asarray(inputs["x"], np.float32)
    Wqkv = np.asarray(inputs["Wqkv"], np.float64)
    bqkv = np.asarray(inputs["bqkv"], np.float64)
    Wout = np.asarray(inputs["Wout"], np.float64)
    bout = np.asarray(inputs["bout"], np.float32)
    Wgate = np.asarray(inputs["Wgate"], np.float64)
    bgate = np.asarray(inputs["bgate"], np.float64)
    scale_gain = np.asarray(inputs["scale_gain"], np.float64)
    skip_w = np.asarray(inputs["skip_w"], np.float64)
    coupling = np.asarray(inputs["coupling"], np.float64)

    gains = _softmax(scale_gain, axis=0)              # [11, H]
    sw = 1.0 / (1.0 + np.exp(-skip_w))                # [2]
    coup = _softmax(coupling, axis=-1)                # [H, H]

    sidx = {s: i for i, s in enumerate(SHIFTS)}
    wtab = np.zeros((len(SHIFTS), H), np.float64)
    for j in range(N_SCALES):
        d = 1 << j
        for t in range(4):
            wtab[sidx[(3 - t) * d]] += D4[t] * gains[j]

    Wq = Wqkv[:, :D]
    Wk = Wqkv[:, D:2 * D]
    Wv = Wqkv[:, 2 * D:]
    Wqg = Wq @ Wgate                                  # folded gate proj
    bg_full = bqkv[:D] @ Wgate + bgate

    postmap = np.concatenate([_locmap(0), _locmap(1)])  # [1024]

    # uniform (parity-independent) tensors
    r = np.arange(128)
    Wg8 = np.zeros((128, KK, KK, 128), NP_F8)
    Wo16 = np.zeros((128, KK, D), NP_BF)
    bgT = np.zeros((128, KK), np.float32)
    for q in range(KK):
        cols = postmap[q * 128 + np.arange(128)]
        for kk in range(KK):
            Wg8[:, kk, q, :] = Wqg[kk * 128:(kk + 1) * 128, cols].astype(NP_F8)
        Wo16[:, q, :] = Wout[cols, :].astype(NP_BF)
        bgT[:, q] = bg_full[cols].astype(np.float32)

    rr = np.arange(128)[:, None]
    mm = np.arange(128)[None, :]
    same_idx = (rr // MYH) == (mm // MYH)
    coup_blk = np.zeros((2, 2, 128, 128), np.float64)
    for jo in range(2):
        for ji in range(2):
            coup_blk[jo, ji] = (coup[MYH * jo + mm % MYH, MYH * ji + rr % MYH]
                                * same_idx)

    bo8 = np.zeros((128, MYH), np.float16)
    bo8[r, r % MYH] = 1.0
    on8 = np.zeros((MYH, 128), NP_BF)
    on8[r % MYH, r] = 1.0
    ident = np.eye(128, dtype=NP_BF)
    swt = np.broadcast_to(sw.astype(np.float32), (128, 2)).copy()
    ones1 = np.ones((1, 128), np.float32)
    bout1 = bout.reshape(1, D).astype(np.float32)

    shared = dict(Wg8=Wg8, Wo16=Wo16, bgT=bgT, bo8_in=bo8,
                  on8_in=on8, ident_in=ident, swt=swt, ones1=ones1,
                  bout1=bout1)

    in_maps = []
    for core in range(NCORES):
        b, g = core // 2, core % 2
        loc = _locmap(g)

        # coupling stationary: gather-half hh contributes only when hh == g
        Mc = np.zeros((128, 2, 2, 2, 128), NP_BF)
        for jo in range(2):
            for ji in range(2):
                Mc[:, jo, g, ji, :] = coup_blk[jo, ji].astype(NP_BF)
        heads = MYH * g + np.arange(128) % MYH        # head of partition r

        Wk8 = np.zeros((128, KK, KC, 128), NP_F8)
        Wv16 = np.zeros((128, KK, KC, 128), np.float16)
        for c in range(KC):
            cols = loc[c * 128 + np.arange(128)]
            for kk in range(KK):
                Wk8[:, kk, c, :] = Wk[kk * 128:(kk + 1) * 128, cols].astype(NP_F8)
                Wv16[:, kk, c, :] = Wv[kk * 128:(kk + 1) * 128, cols].astype(np.float16)
        bqkT = bqkv[D + loc].reshape(KC, 128).T.copy().astype(np.float32)
        bvT = bqkv[2 * D + loc].reshape(KC, 128).T.copy().astype(np.float32)

        Md = np.zeros((128, len(PE_TAPS), 128), NP_BF)
        for ti, s in enumerate(PE_TAPS):
            Md[r, ti, r] = wtab[sidx[s], heads].astype(NP_BF)
        wew = np.zeros((128, len(SC_TAPS)), np.float32)
        for i, s in enumerate(SC_TAPS):
            wew[:, i] = wtab[sidx[s], heads].astype(np.float32)
        wdv = np.zeros((128, len(DV_TAPS), 1024), NP_BF)
        for i, s in enumerate(DV_TAPS):
            wdv[:, i, :] = np.broadcast_to(
                wtab[sidx[s], heads].astype(NP_BF)[:, None], (128, 1024))

        xb = x[b]                                     # [N, D]
        x16T = np.ascontiguousarray(
            xb.T.reshape(KK, 128, N).transpose(1, 0, 2)).astype(np.float16)
        x8T = x16T.astype(NP_F8)
        xg = xb[g * SEQ:(g + 1) * SEQ, :]             # [SEQ, D]
        xg8 = np.ascontiguousarray(
            xg.T.reshape(KK, 128, SEQ).transpose(1, 0, 2)).astype(NP_F8)

        in_maps.append(dict(x16T=x16T, x8T=x8T, xg8=xg8, Wk8=Wk8, Wv16=Wv16,
                            Mdiag=Md, wtap_ew=wew, Wdv=wdv, bqkT=bqkT,
                            bvT=bvT, Mcoup=Mc, **shared))
    return in_maps


def run_cores(inputs, debug_outputs=False, trace=False):
    nc = _build_program(debug_outputs=debug_outputs)
    in_maps = _host_prep(inputs)
    res = run_bass_kernel_spmd(nc, in_maps, list(range(NCORES)), trace=trace)
    return res


def kernel(**inputs) -> np.ndarray:
    res = run_cores(inputs)
    out = np.empty((B, N, D), np.float32)
    for c in range(NCORES):
        b, g = c // 2, c % 2
        out[b, g * SEQ:(g + 1) * SEQ, :] = res.results[c]["out"]
    return out


# revision 37
# speedup vs baseline: 1.2665x; 1.0175x over previous
"""Trainium2 Bass kernel for CausalWaveletFieldAttention (v4).

Sharding: (batch, head-half). Core c = (b = c//2, g = c%2) owns global
heads [8g, 8g+8) (512 channels) for the FULL 4096-token sequence, so the
causal wavelet conv needs no halo and no mid-kernel collectives. After
conv+skip, ONE AllToAll per 128-channel chunk exchanges field halves
(core keeps seq rows [2048g, 2048g+2048) of all 1024 channels); the
AllToAll slot index equals the source core parity, so the post-exchange
channel layout is identical on both cores and all tail weights are
uniform. Core c writes output rows [2048g, 2048g+2048) of batch b.

Per-core channel layout ("local"): position p in [0,512): chunk c=p//128,
local head h'=p%8 (global head 8g+h'), idx = 16c + (p%128)//8.
Post-exchange position P in [0,1024): slot j=P//512 (= source parity),
then local map with g=j.

Engine plan:
  - k-proj + gate-proj in fp8 DoubleRow (2 contraction chunks/matmul),
    v-proj fp16, out-proj bf16 (fp8 fails the error budget there).
  - kmag via block-ones matmul accumulated across chunks (psum held
    open), broadcast back with a [8,128] ones matmul.
  - conv: 13 large-shift taps as PE diagonal matmuls with range
    splitting (zero history is never materialized beyond a 128-col pad);
    11 small-shift taps as ScalarE scale-copies + DVE/GpSimd add chain;
    chain merged into the conv PSUM with an identity matmul.
  - skip taps on ScalarE (scale) + DVE (adds), pre-exchange, full seq.
  - coupling as 2 block-diag [128,128] matmuls per output chunk
    (contraction only over the 16 heads at equal idx).
  - gate kept in SBUF (no DRAM round trip), multiplied in by DVE.
"""

import ml_dtypes
import numpy as np

import concourse.bass as bass
import concourse.mybir as mybir
import concourse.tile as tile
from concourse import bacc
from concourse.bass_utils import run_bass_kernel_spmd

F32 = mybir.dt.float32
F16 = mybir.dt.float16
BF16 = mybir.dt.bfloat16
F8 = mybir.dt.float8e4
AF = mybir.ActivationFunctionType
DR = mybir.MatmulPerfMode.DoubleRow

NP_F8 = ml_dtypes.float8_e4m3fn
NP_BF = ml_dtypes.bfloat16

B, N, D, H, HD = 4, 4096, 1024, 16, 64
NCORES = 8
SEQ = N // 2            # rows per core in the tail phases
MYH = 8                 # heads per core
CH = 512                # channels per core
KC = CH // 128          # 4 local chunks
KK = D // 128           # 8 contraction chunks
PAD = 128               # zero pad in front of f0 for small-shift taps

D4 = [0.4829629131445341, 0.8365163037378079, 0.2241438680420134, -0.1294095225512604]
N_SCALES = 11
SHIFTS = [0, 1, 2, 3, 4, 6, 8, 12, 16, 24, 32, 48, 64, 96, 128, 192, 256,
          384, 512, 768, 1024, 1536, 2048, 3072]
PE_TAPS = [0, 24, 32, 48, 64, 96, 128, 192, 256, 384, 512, 768, 1024,
           1536, 2048, 3072]
SC_TAPS = [1, 3, 16]              # ScalarE scale-copy (1x mode, ~2.1us/2048)
DV_TAPS = [2, 4, 6, 8, 12]        # DVE TT-mul vs broadcast weights (2x mode)
GROUPS = [[0, 1], [2, 3], [4, 5], [6, 7]]

_PROGRAM_CACHE = {}


def _build_program(debug_outputs=False):
    key = bool(debug_outputs)
    if key in _PROGRAM_CACHE:
        return _PROGRAM_CACHE[key]

    nc = bacc.Bacc("TRN2", target_bir_lowering=False, debug=False,
                   num_devices=NCORES)

    # ---- DRAM parameters (per-core) ----
    x16T = nc.declare_dram_parameter("x16T", [128, KK, N], F16, isOutput=False)
    x8T = nc.declare_dram_parameter("x8T", [128, KK, N], F8, isOutput=False)
    xg8 = nc.declare_dram_parameter("xg8", [128, KK, SEQ], F8, isOutput=False)
    Wk8 = nc.declare_dram_parameter("Wk8", [128, KK, KC, 128], F8, isOutput=False)
    Wv16 = nc.declare_dram_parameter("Wv16", [128, KK, KC, 128], F16, isOutput=False)
    Wg8 = nc.declare_dram_parameter("Wg8", [128, KK, KK, 128], F8, isOutput=False)
    Wo16 = nc.declare_dram_parameter("Wo16", [128, KK, D], BF16, isOutput=False)
    Mdiag = nc.declare_dram_parameter("Mdiag", [128, len(PE_TAPS), 128], BF16,
                                      isOutput=False)
    Mcoup = nc.declare_dram_parameter("Mcoup", [128, 2, 2, 2, 128], BF16,
                                      isOutput=False)
    ident_in = nc.declare_dram_parameter("ident_in", [128, 128], BF16,
                                         isOutput=False)
    bo8_in = nc.declare_dram_parameter("bo8_in", [128, MYH], F16, isOutput=False)
    on8_in = nc.declare_dram_parameter("on8_in", [MYH, 128], BF16, isOutput=False)
    wtap_ew = nc.declare_dram_parameter("wtap_ew", [128, len(SC_TAPS)], F32,
                                        isOutput=False)
    Wdv = nc.declare_dram_parameter("Wdv", [128, len(DV_TAPS), 1024], BF16,
                                    isOutput=False)
    ones1 = nc.declare_dram_parameter("ones1", [1, 128], mybir.dt.float32r,
                                      isOutput=False)
    bout1 = nc.declare_dram_parameter("bout1", [1, D], mybir.dt.float32r,
                                      isOutput=False)
    swt = nc.declare_dram_parameter("swt", [128, 2], F32, isOutput=False)
    bqkT = nc.declare_dram_parameter("bqkT", [128, KC], F32, isOutput=False)
    bvT = nc.declare_dram_parameter("bvT", [128, KC], F32, isOutput=False)
    bgT = nc.declare_dram_parameter("bgT", [128, KK], F32, isOutput=False)
    out = nc.declare_dram_parameter("out", [SEQ, D], F32, isOutput=True)

    dbg = {}
    if debug_outputs:
        for name, shape in (("dbg_f0", [128, KC, N]),
                            ("dbg_conv", [128, KC, N]),
                            ("dbg_field", [128, KC, N]),
                            ("dbg_km", [MYH, N]),
                            ("dbg_gate", [128, KK, SEQ]),
                            ("dbg_pgs", [128, KK, SEQ])):
            dbg[name] = nc.declare_dram_parameter(name, shape, BF16,
                                                  isOutput=True)

    # ---- internal DRAM for the exchange (one pairwise AllGather/chunk) ----
    ag_in = [nc.dram_tensor(f"ag_in{c}", [128, N], BF16) for c in range(KC)]
    ag_out = [nc.dram_tensor(f"ag_out{c}", [2, 128, N], BF16)
              for c in range(KC)]

    with tile.TileContext(nc) as tc:
        with (
            tc.tile_pool(name="const", bufs=1) as constp,
            tc.tile_pool(name="p_long", bufs=1) as p_long,
        ):
            # ---- always-resident constants ----
            md_t = constp.tile([128, len(PE_TAPS), 128], BF16)
            nc.sync.dma_start(md_t[:], Mdiag[:])
            mc_t = constp.tile([128, 2, 2, 2, 128], BF16)
            nc.sync.dma_start(mc_t[:], Mcoup[:])
            id_t = constp.tile([128, 128], BF16)
            nc.sync.dma_start(id_t[:], ident_in[:])
            bo_t = constp.tile([128, MYH], F16)
            nc.sync.dma_start(bo_t[:], bo8_in[:])
            on_t = constp.tile([MYH, 128], BF16)
            nc.sync.dma_start(on_t[:], on8_in[:])
            wew_t = constp.tile([128, len(SC_TAPS)], F32)
            nc.sync.dma_start(wew_t[:], wtap_ew[:])
            wdv_t = constp.tile([128, len(DV_TAPS), 1024], BF16)
            nc.sync.dma_start(wdv_t[:], Wdv[:])
            on1_t = constp.tile([1, 128], mybir.dt.float32r)
            nc.sync.dma_start(on1_t[:], ones1[:])
            bo1_t = constp.tile([1, D], mybir.dt.float32r)
            nc.sync.dma_start(bo1_t[:], bout1[:])
            swt_t = constp.tile([128, 2], F32)
            nc.sync.dma_start(swt_t[:], swt[:])
            bqk_t = constp.tile([128, KC], F32)
            nc.sync.dma_start(bqk_t[:], bqkT[:])
            bv_t = constp.tile([128, KC], F32)
            nc.sync.dma_start(bv_t[:], bvT[:])
            bg_t = constp.tile([128, KK], F32)
            nc.sync.dma_start(bg_t[:], bgT[:])

            km_sb = p_long.tile([MYH, N], BF16, tag="km_sb")
            kmb = p_long.tile([128, N], BF16, tag="kmb")

            with tc.tile_pool(name="px16", bufs=1) as px16:
                # allocate first, DMA after the K-phase inputs are queued
                x16 = px16.tile([128, KK, N], F16, tag="x16")
                wv_t = px16.tile([128, KK, KC, 128], F16, tag="wv_t")

                # ================= K phase: kmag =========================
                with (
                    tc.tile_pool(name="px8", bufs=1) as px8,
                    tc.tile_pool(name="pwk", bufs=2) as pwk,
                ):
                    x8 = px8.tile([128, KK, N], F8, tag="x8")
                    wk_t = px8.tile([128, KK, KC, 128], F8, tag="wk_t")
                    nc.sync.dma_start(wk_t[:], Wk8[:])
                    for half in range(2):
                        h0 = half * SEQ
                        for jp in range(KK // 2):
                            nc.sync.dma_start(
                                x8[:, 2 * jp:2 * jp + 2, h0:h0 + SEQ],
                                x8T[:, 2 * jp:2 * jp + 2, h0:h0 + SEQ])
                    # now the big v-phase loads (overlap with K compute)
                    nc.sync.dma_start(x16[:], x16T[:])
                    nc.sync.dma_start(wv_t[:], Wv16[:])

                    with (
                        tc.tile_pool(name="psk", bufs=1, space="PSUM") as psk,
                        tc.tile_pool(name="pskm", bufs=1, space="PSUM") as pskm,
                    ):
                        for half in range(2):
                            h0 = half * SEQ
                            kms = [pskm.tile([MYH, 512], F32, name=f"km{i}",
                                             tag=f"km{i}") for i in range(4)]
                            for c in range(KC):
                                kps = [psk.tile([128, 512], F32, name=f"kp{i}",
                                                tag=f"kp{i}") for i in range(4)]
                                for jp in range(KK // 2):
                                    for bk in range(4):
                                        nc.tensor.matmul(
                                            kps[bk][:],
                                            wk_t[:, 2 * jp:2 * jp + 2, c, :],
                                            x8[:, 2 * jp:2 * jp + 2,
                                               h0 + bk * 512:h0 + bk * 512 + 512],
                                            start=(jp == 0), stop=(jp == 3),
                                            perf_mode=DR)
                                k2t = pwk.tile([128, SEQ], F16, tag="k2t")
                                for bk in range(4):
                                    nc.scalar.activation(
                                        k2t[:, bk * 512:(bk + 1) * 512],
                                        kps[bk][:],
                                        AF.Square, bias=bqk_t[:, c:c + 1])
                                for bk in range(4):
                                    nc.tensor.matmul(
                                        kms[bk][:], bo_t[:],
                                        k2t[:, bk * 512:(bk + 1) * 512],
                                        start=(c == 0), stop=(c == KC - 1))
                            for bk in range(4):
                                nc.scalar.activation(
                                    km_sb[:, h0 + bk * 512:h0 + bk * 512 + 512],
                                    kms[bk][:], AF.Sqrt)

                    # kmag broadcast to all 128 partitions (same all chunks)
                    with tc.tile_pool(name="pskb", bufs=1,
                                      space="PSUM") as pskb:
                        kbs = [pskb.tile([128, 512], F32, name=f"kb{i}",
                                         tag=f"kb{i}") for i in range(4)]
                        for half in range(2):
                            h0 = half * SEQ
                            for bk in range(4):
                                nc.tensor.matmul(
                                    kbs[bk][:], on_t[:],
                                    km_sb[:, h0 + bk * 512:h0 + bk * 512 + 512],
                                    start=True, stop=True)
                            for bk in range(4):
                                nc.scalar.activation(
                                    kmb[:, h0 + bk * 512:h0 + bk * 512 + 512],
                                    kbs[bk][:], AF.Identity)
                if debug_outputs:
                    nc.sync.dma_start(dbg["dbg_km"][:], km_sb[:])

                # ============ V + conv pipeline ==========================
                with (
                    tc.tile_pool(name="pf0", bufs=2) as pf0,
                    tc.tile_pool(name="pwv", bufs=2) as pwv,
                    tc.tile_pool(name="pat", bufs=3) as pat,
                    tc.tile_pool(name="pacc", bufs=2) as pacc,
                    tc.tile_pool(name="pcb", bufs=2) as pcb,
                    tc.tile_pool(name="pcs", bufs=2) as pcs,
                    tc.tile_pool(name="pfl", bufs=1) as pfl,
                    tc.tile_pool(name="psv", bufs=1, space="PSUM") as psv,
                    tc.tile_pool(name="psc", bufs=1, space="PSUM") as psc,
                ):
                    f0ts = {}
                    cbts = {}

                    def v_work(c):
                        f0t = pf0.tile([128, PAD + N], BF16, tag="f0t")
                        f0ts[c] = f0t
                        if c < 2:
                            nc.gpsimd.memset(f0t[:, 0:PAD], 0.0)
                        for half in range(2):
                            h0 = half * SEQ
                            vps = [psv.tile([128, 512], F32, name=f"vp{i}",
                                            tag=f"vp{i}") for i in range(4)]
                            for kk in range(KK):
                                for bk in range(4):
                                    nc.tensor.matmul(
                                        vps[bk][:], wv_t[:, kk, c, :],
                                        x16[:, kk,
                                            h0 + bk * 512:h0 + bk * 512 + 512],
                                        start=(kk == 0), stop=(kk == KK - 1))
                            vbt = pwv.tile([128, SEQ], BF16, tag="vbt")
                            for bk in range(4):
                                nc.scalar.activation(
                                    vbt[:, bk * 512:(bk + 1) * 512], vps[bk][:],
                                    AF.Identity, bias=bv_t[:, c:c + 1])
                            nc.vector.tensor_mul(
                                f0t[:, PAD + h0:PAD + h0 + SEQ], vbt[:],
                                kmb[:, h0:h0 + SEQ])
                        if debug_outputs:
                            nc.sync.dma_start(dbg["dbg_f0"][:, c, :],
                                              f0t[:, PAD:])

                    def conv_ew(c):
                        """small-shift taps: DVE TT-mul (broadcast weights)
                        and ScalarE scale-copy, then DVE/GpSimd add chain."""
                        f0t = f0ts[c]
                        cbt = pcb.tile([128, N], BF16, tag="cbt")
                        cbts[c] = cbt
                        n_terms = len(DV_TAPS) + len(SC_TAPS)
                        for half in range(2):
                            base = PAD + half * SEQ
                            acc = None
                            nadd = 0
                            for i in range(n_terms):
                                att = pat.tile([128, SEQ], BF16, tag="att")
                                if i < len(DV_TAPS):
                                    s = DV_TAPS[i]
                                    for hb in range(2):
                                        o = base - s + hb * 1024
                                        nc.vector.tensor_mul(
                                            att[:, hb * 1024:(hb + 1) * 1024],
                                            f0t[:, o:o + 1024],
                                            wdv_t[:, i, :])
                                else:
                                    j = i - len(DV_TAPS)
                                    s = SC_TAPS[j]
                                    nc.scalar.activation(
                                        att[:], f0t[:, base - s:base - s + SEQ],
                                        AF.Identity, scale=wew_t[:, j:j + 1])
                                if acc is None:
                                    acc = att[:]
                                    continue
                                nadd += 1
                                if i == n_terms - 1:
                                    nxt = cbt[:, half * SEQ:(half + 1) * SEQ]
                                else:
                                    acct = pacc.tile([128, SEQ], BF16,
                                                     tag="acc", name="acct")
                                    nxt = acct[:]
                                if nadd == 3:
                                    nc.gpsimd.tensor_add(nxt, acc, att[:])
                                else:
                                    nc.vector.tensor_add(nxt, acc, att[:])
                                acc = nxt

                    def conv_pe(c):
                        """PE diag taps + chain merge + evict + skip + a2a."""
                        f0t = f0ts.pop(c)
                        cbt = cbts.pop(c)
                        cst = pcs.tile([128, N], BF16, tag="cst")
                        for chalf in range(2):
                            cps = [psc.tile([128, 512], F32, name=f"cp{i}",
                                            tag=f"cp{i}") for i in range(4)]
                            for ti, s in enumerate(PE_TAPS):
                                for bk in range(4):
                                    bg = chalf * 4 + bk
                                    lo = max(0, s - PAD - 512 * bg)
                                    if lo >= 512:
                                        continue
                                    src0 = PAD + 512 * bg + lo - s
                                    nc.tensor.matmul(
                                        cps[bk][:, lo:512], md_t[:, ti, :],
                                        f0t[:, src0:src0 + 512 - lo],
                                        start=(ti == 0), stop=False)
                            for bk in range(4):
                                bg = chalf * 4 + bk
                                nc.tensor.matmul(
                                    cps[bk][:], id_t[:],
                                    cbt[:, bg * 512:(bg + 1) * 512],
                                    start=False, stop=True)
                            for bk in range(4):
                                bg = chalf * 4 + bk
                                nc.scalar.activation(
                                    cst[:, bg * 512:(bg + 1) * 512],
                                    cps[bk][:], AF.Identity)
                        if debug_outputs:
                            nc.sync.dma_start(dbg["dbg_conv"][:, c, :], cst[:])
                        # skip taps: field = conv + sw0*conv[-512] + sw1*conv[-1024]
                        # temporaries borrow the pat/pacc rotating buffers
                        flt = pfl.tile([128, N], BF16, tag="flt")
                        t0a = pat.tile([128, SEQ], BF16, tag="att", name="t0a")
                        t1a = pat.tile([128, SEQ], BF16, tag="att", name="t1a")
                        nc.scalar.activation(t0a[:], cst[:, 0:SEQ],
                                             AF.Identity, scale=swt_t[:, 0:1])
                        nc.scalar.activation(t1a[:], cst[:, 0:SEQ],
                                             AF.Identity, scale=swt_t[:, 1:2])
                        ua = pacc.tile([128, 1024], BF16, tag="ua")
                        nc.vector.tensor_copy(flt[:, 0:512], cst[:, 0:512])
                        nc.vector.tensor_add(flt[:, 512:1024],
                                             cst[:, 512:1024], t0a[:, 0:512])
                        nc.vector.tensor_add(ua[:], t0a[:, 512:1536],
                                             t1a[:, 0:1024])
                        nc.vector.tensor_add(flt[:, 1024:2048],
                                             cst[:, 1024:2048], ua[:])
                        t0b = pat.tile([128, SEQ], BF16, tag="att", name="t0b")
                        t1b = pat.tile([128, SEQ], BF16, tag="att", name="t1b")
                        nc.scalar.activation(t0b[:], cst[:, 1536:1536 + SEQ],
                                             AF.Identity, scale=swt_t[:, 0:1])
                        nc.scalar.activation(t1b[:], cst[:, 1024:1024 + SEQ],
                                             AF.Identity, scale=swt_t[:, 1:2])
                        ub = pacc.tile([128, SEQ], BF16, tag="acc", name="ub")
                        nc.vector.tensor_add(ub[:], t0b[:], t1b[:])
                        nc.vector.tensor_add(flt[:, SEQ:], cst[:, SEQ:],
                                             ub[:])
                        if debug_outputs:
                            nc.sync.dma_start(dbg["dbg_field"][:, c, :], flt[:])
                        nc.sync.dma_start(ag_in[c][:], flt[:])
                        nc.gpsimd.collective_compute(
                            "AllGather", mybir.AluOpType.bypass,
                            replica_groups=GROUPS,
                            ins=[ag_in[c][:]], outs=[ag_out[c][:]])

                    for c in range(KC + 1):
                        if c < KC:
                            v_work(c)
                            conv_ew(c)
                        if c >= 1:
                            conv_pe(c - 1)

            # ================= tail: gate, couple, out ===================
            with (
                tc.tile_pool(name="ptail", bufs=1) as ptail,
                tc.tile_pool(name="pgt", bufs=2) as pgt,
                tc.tile_pool(name="pff", bufs=2) as pff,
                tc.tile_pool(name="pob", bufs=2) as pob,
            ):
                xg = ptail.tile([128, KK, SEQ], F8, tag="xg")
                nc.sync.dma_start(xg[:], xg8[:])
                wg_t = ptail.tile([128, KK, KK, 128], F8, tag="wg_t")
                nc.sync.dma_start(wg_t[:], Wg8[:])
                wo_t = ptail.tile([128, KK, D], BF16, tag="wo_t")
                nc.sync.dma_start(wo_t[:], Wo16[:])
                gate = ptail.tile([128, KK, SEQ], BF16, tag="gate")
                pgs = ptail.tile([128, KK, SEQ], BF16, tag="pgs")

                with tc.tile_pool(name="pscp", bufs=1, space="PSUM") as pscp:
                    with tc.tile_pool(name="psg", bufs=1, space="PSUM") as psg:
                        # gate = sigmoid(x @ (Wq@Wgate) + b'), fp8 DoubleRow
                        for q in range(KK):
                            gps = [psg.tile([128, 512], F32, name=f"gp{i}",
                                            tag=f"gp{i}") for i in range(4)]
                            for jp in range(KK // 2):
                                for bk in range(4):
                                    nc.tensor.matmul(
                                        gps[bk][:],
                                        wg_t[:, 2 * jp:2 * jp + 2, q, :],
                                        xg[:, 2 * jp:2 * jp + 2,
                                           bk * 512:bk * 512 + 512],
                                        start=(jp == 0), stop=(jp == 3),
                                        perf_mode=DR)
                            for bk in range(4):
                                nc.scalar.activation(
                                    gate[:, q, bk * 512:(bk + 1) * 512],
                                    gps[bk][:],
                                    AF.Sigmoid, bias=bg_t[:, q:q + 1])
                        if debug_outputs:
                            nc.sync.dma_start(dbg["dbg_gate"][:], gate[:])

                        # coupling + gate multiply, per exchange chunk.
                        # mc_t[:, jo, hh, ji, :] is host-zeroed unless hh
                        # equals this core's parity, selecting which gather
                        # half (L/R) feeds the couple without runtime offsets.
                        for c in range(KC):
                            fft = pff.tile([128, 2, N], BF16, tag="fft")
                            nc.sync.dma_start(
                                fft[:],
                                ag_out[c][:].rearrange("j p n -> p j n"))
                            for jo in range(2):
                                q = jo * KC + c
                                cpp = [pscp.tile([128, 512], F32,
                                                 name=f"cc{i}", tag=f"cc{i}")
                                       for i in range(4)]
                                for bk in range(4):
                                    first = True
                                    for hh in range(2):
                                        for ji in range(2):
                                            nc.tensor.matmul(
                                                cpp[bk][:],
                                                mc_t[:, jo, hh, ji, :],
                                                fft[:, ji, hh * SEQ + bk * 512:
                                                    hh * SEQ + (bk + 1) * 512],
                                                start=first,
                                                stop=(hh == 1 and ji == 1))
                                            first = False
                                for bk in range(4):
                                    nc.vector.tensor_mul(
                                        pgs[:, q, bk * 512:(bk + 1) * 512],
                                        cpp[bk][:],
                                        gate[:, q, bk * 512:(bk + 1) * 512])
                        if debug_outputs:
                            nc.sync.dma_start(dbg["dbg_pgs"][:], pgs[:])

                    # out = pgs.T @ Wout + bout (bias via rank-1 f32r matmul,
                    # result DMAed PSUM -> DRAM directly)
                    with tc.tile_pool(name="pso", bufs=2, space="PSUM") as pso:
                        for st in range(SEQ // 128):
                            ops = [pso.tile([128, 512], F32, name=f"op{i}",
                                            tag=f"op{i}") for i in range(2)]
                            for cb in range(2):
                                nc.tensor.matmul(
                                    ops[cb][:], on1_t[:],
                                    bo1_t[:, cb * 512:(cb + 1) * 512],
                                    start=True, stop=False)
                            for q in range(KK):
                                for cb in range(2):
                                    nc.tensor.matmul(
                                        ops[cb][:],
                                        pgs[:, q, st * 128:(st + 1) * 128],
                                        wo_t[:, q, cb * 512:(cb + 1) * 512],
                                        start=False, stop=(q == KK - 1))
                            outb = pob.tile([128, D], F32, tag="outb")
                            for cb in range(2):
                                nc.vector.tensor_copy(
                                    outb[:, cb * 512:(cb + 1) * 512],
                                    ops[cb][:])
                            nc.sync.dma_start(out[st * 128:(st + 1) * 128, :],
                                              outb[:])

    nc.compile()
    _PROGRAM_CACHE[key] = nc
    return nc


def _softmax(a, axis):
    a = a - a.max(axis=axis, keepdims=True)
    e = np.exp(a)
    return e / e.sum(axis=axis, keepdims=True)


def _locmap(g):
    """local position p (0..511) -> original channel index (0..1023)."""
    p = np.arange(CH)
    return (MYH * g + p % MYH) * HD + 16 * (p // 128) + (p % 128) // MYH


def _host_prep(inputs):
    x = np.asarray(inputs["x"], np.float32)
    Wqkv = np.asarray(inputs["Wqkv"], np.float64)
    bqkv = np.asarray(inputs["bqkv"], np.float64)
    Wout = np.asarray(inputs["Wout"], np.float64)
    bout = np.asarray(inputs["bout"], np.float32)
    Wgate = np.asarray(inputs["Wgate"], np.float64)
    bgate = np.asarray(inputs["bgate"], np.float64)
    scale_gain = np.asarray(inputs["scale_gain"], np.float64)
    skip_w = np.asarray(inputs["skip_w"], np.float64)
    coupling = np.asarray(inputs["coupling"], np.float64)

    gains = _softmax(scale_gain, axis=0)              # [11, H]
    sw = 1.0 / (1.0 + np.exp(-skip_w))                # [2]
    coup = _softmax(coupling, axis=-1)                # [H, H]

    sidx = {s: i for i, s in enumerate(SHIFTS)}
    wtab = np.zeros((len(SHIFTS), H), np.float64)
    for j in range(N_SCALES):
        d = 1 << j
        for t in range(4):
            wtab[sidx[(3 - t) * d]] += D4[t] * gains[j]

    Wq = Wqkv[:, :D]
    Wk = Wqkv[:, D:2 * D]
    Wv = Wqkv[:, 2 * D:]
    Wqg = Wq @ Wgate                                  # folded gate proj
    bg_full = bqkv[:D] @ Wgate + bgate

    postmap = np.concatenate([_locmap(0), _locmap(1)])  # [1024]

    # uniform (parity-independent) tensors
    r = np.arange(128)
    Wg8 = np.zeros((128, KK, KK, 128), NP_F8)
    Wo16 = np.zeros((128, KK, D), NP_BF)
    bgT = np.zeros((128, KK), np.float32)
    for q in range(KK):
        cols = postmap[q * 128 + np.arange(128)]
        for kk in range(KK):
            Wg8[:, kk, q, :] = Wqg[kk * 128:(kk + 1) * 128, cols].astype(NP_F8)
        Wo16[:, q, :] = Wout[cols, :].astype(NP_BF)
        bgT[:, q] = bg_full[cols].astype(np.float32)

    rr = np.arange(128)[:, None]
    mm = np.arange(128)[None, :]
    same_idx = (rr // MYH) == (mm // MYH)
    coup_blk = np.zeros((2, 2, 128, 128), np.float64)
    for jo in range(2):
        for ji in range(2):
            coup_blk[jo, ji] = (coup[MYH * jo + mm % MYH, MYH * ji + rr % MYH]
                                * same_idx)

    bo8 = np.zeros((128, MYH), np.float16)
    bo8[r, r % MYH] = 1.0
    on8 = np.zeros((MYH, 128), NP_BF)
    on8[r % MYH, r] = 1.0
    ident = np.eye(128, dtype=NP_BF)
    swt = np.broadcast_to(sw.astype(np.float32), (128, 2)).copy()
    ones1 = np.ones((1, 128), np.float32)
    bout1 = bout.reshape(1, D).astype(np.float32)

    shared = dict(Wg8=Wg8, Wo16=Wo16, bgT=bgT, bo8_in=bo8,
                  on8_in=on8, ident_in=ident, swt=swt, ones1=ones1,
                  bout1=bout1)

    in_maps = []
    for core in range(NCORES):
        b, g = core // 2, core % 2
        loc = _locmap(g)

        # coupling stationary: gather-half hh contributes only when hh == g
        Mc = np.zeros((128, 2, 2, 2, 128), NP_BF)
        for jo in range(2):
            for ji in range(2):
                Mc[:, jo, g, ji, :] = coup_blk[jo, ji].astype(NP_BF)
        heads = MYH * g + np.arange(128) % MYH        # head of partition r

        Wk8 = np.zeros((128, KK, KC, 128), NP_F8)
        Wv16 = np.zeros((128, KK, KC, 128), np.float16)
        for c in range(KC):
            cols = loc[c * 128 + np.arange(128)]
            for kk in range(KK):
                Wk8[:, kk, c, :] = Wk[kk * 128:(kk + 1) * 128, cols].astype(NP_F8)
                Wv16[:, kk, c, :] = Wv[kk * 128:(kk + 1) * 128, cols].astype(np.float16)
        bqkT = bqkv[D + loc].reshape(KC, 128).T.copy().astype(np.float32)
        bvT = bqkv[2 * D + loc].reshape(KC, 128).T.copy().astype(np.float32)

        Md = np.zeros((128, len(PE_TAPS), 128), NP_BF)
        for ti, s in enumerate(PE_TAPS):
            Md[r, ti, r] = wtab[sidx[s], heads].astype(NP_BF)
        wew = np.zeros((128, len(SC_TAPS)), np.float32)
        for i, s in enumerate(SC_TAPS):
            wew[:, i] = wtab[sidx[s], heads].astype(np.float32)
        wdv = np.zeros((128, len(DV_TAPS), 1024), NP_BF)
        for i, s in enumerate(DV_TAPS):
            wdv[:, i, :] = np.broadcast_to(
                wtab[sidx[s], heads].astype(NP_BF)[:, None], (128, 1024))

        xb = x[b]                                     # [N, D]
        x16T = np.ascontiguousarray(
            xb.T.reshape(KK, 128, N).transpose(1, 0, 2)).astype(np.float16)
        x8T = x16T.astype(NP_F8)
        xg = xb[g * SEQ:(g + 1) * SEQ, :]             # [SEQ, D]
        xg8 = np.ascontiguousarray(
            xg.T.reshape(KK, 128, SEQ).transpose(1, 0, 2)).astype(NP_F8)

        in_maps.append(dict(x16T=x16T, x8T=x8T, xg8=xg8, Wk8=Wk8, Wv16=Wv16,
                            Mdiag=Md, wtap_ew=wew, Wdv=wdv, bqkT=bqkT,
                            bvT=bvT, Mcoup=Mc, **shared))
    return in_maps


def run_cores(inputs, debug_outputs=False, trace=False):
    nc = _build_program(debug_outputs=debug_outputs)
    in_maps = _host_prep(inputs)
    res = run_bass_kernel_spmd(nc, in_maps, list(range(NCORES)), trace=trace)
    return res


def kernel(**inputs) -> np.ndarray:
    res = run_cores(inputs)
    out = np.empty((B, N, D), np.float32)
    for c in range(NCORES):
        b, g = c // 2, c % 2
        out[b, g * SEQ:(g + 1) * SEQ, :] = res.results[c]["out"]
    return out


# revision 38
# speedup vs baseline: 1.2712x; 1.0037x over previous
"""Trainium2 Bass kernel for CausalWaveletFieldAttention (v4).

Sharding: (batch, head-half). Core c = (b = c//2, g = c%2) owns global
heads [8g, 8g+8) (512 channels) for the FULL 4096-token sequence, so the
causal wavelet conv needs no halo and no mid-kernel collectives. After
conv+skip, ONE AllToAll per 128-channel chunk exchanges field halves
(core keeps seq rows [2048g, 2048g+2048) of all 1024 channels); the
AllToAll slot index equals the source core parity, so the post-exchange
channel layout is identical on both cores and all tail weights are
uniform. Core c writes output rows [2048g, 2048g+2048) of batch b.

Per-core channel layout ("local"): position p in [0,512): chunk c=p//128,
local head h'=p%8 (global head 8g+h'), idx = 16c + (p%128)//8.
Post-exchange position P in [0,1024): slot j=P//512 (= source parity),
then local map with g=j.

Engine plan:
  - k-proj + gate-proj in fp8 DoubleRow (2 contraction chunks/matmul),
    v-proj fp16, out-proj bf16 (fp8 fails the error budget there).
  - kmag via block-ones matmul accumulated across chunks (psum held
    open), broadcast back with a [8,128] ones matmul.
  - conv: 13 large-shift taps as PE diagonal matmuls with range
    splitting (zero history is never materialized beyond a 128-col pad);
    11 small-shift taps as ScalarE scale-copies + DVE/GpSimd add chain;
    chain merged into the conv PSUM with an identity matmul.
  - skip taps on ScalarE (scale) + DVE (adds), pre-exchange, full seq.
  - coupling as 2 block-diag [128,128] matmuls per output chunk
    (contraction only over the 16 heads at equal idx).
  - gate kept in SBUF (no DRAM round trip), multiplied in by DVE.
"""

import ml_dtypes
import numpy as np

import concourse.bass as bass
import concourse.mybir as mybir
import concourse.tile as tile
from concourse import bacc
from concourse.bass_utils import run_bass_kernel_spmd

F32 = mybir.dt.float32
F16 = mybir.dt.float16
BF16 = mybir.dt.bfloat16
F8 = mybir.dt.float8e4
AF = mybir.ActivationFunctionType
DR = mybir.MatmulPerfMode.DoubleRow

NP_F8 = ml_dtypes.float8_e4m3fn
NP_BF = ml_dtypes.bfloat16

B, N, D, H, HD = 4, 4096, 1024, 16, 64
NCORES = 8
SEQ = N // 2            # rows per core in the tail phases
MYH = 8                 # heads per core
CH = 512                # channels per core
KC = CH // 128          # 4 local chunks
KK = D // 128           # 8 contraction chunks
PAD = 128               # zero pad in front of f0 for small-shift taps

D4 = [0.4829629131445341, 0.8365163037378079, 0.2241438680420134, -0.1294095225512604]
N_SCALES = 11
SHIFTS = [0, 1, 2, 3, 4, 6, 8, 12, 16, 24, 32, 48, 64, 96, 128, 192, 256,
          384, 512, 768, 1024, 1536, 2048, 3072]
PE_TAPS = [0, 24, 32, 48, 64, 96, 128, 192, 256, 384, 512, 768, 1024,
           1536, 2048, 3072]
SC_TAPS = [1, 3, 16]              # ScalarE scale-copy (1x mode, ~2.1us/2048)
DV_TAPS = [2, 4, 6, 8, 12]        # DVE TT-mul vs broadcast weights (2x mode)
GROUPS = [[0, 1], [2, 3], [4, 5], [6, 7]]

_PROGRAM_CACHE = {}


def _build_program(debug_outputs=False):
    key = bool(debug_outputs)
    if key in _PROGRAM_CACHE:
        return _PROGRAM_CACHE[key]

    nc = bacc.Bacc("TRN2", target_bir_lowering=False, debug=False,
                   num_devices=NCORES)

    # ---- DRAM parameters (per-core) ----
    x16T = nc.declare_dram_parameter("x16T", [128, KK, N], F16, isOutput=False)
    x8T = nc.declare_dram_parameter("x8T", [128, KK, N], F8, isOutput=False)
    xg8 = nc.declare_dram_parameter("xg8", [128, KK, SEQ], F8, isOutput=False)
    Wk8 = nc.declare_dram_parameter("Wk8", [128, KK, KC, 128], F8, isOutput=False)
    Wv16 = nc.declare_dram_parameter("Wv16", [128, KK, KC, 128], F16, isOutput=False)
    Wg8 = nc.declare_dram_parameter("Wg8", [128, KK, KK, 128], F8, isOutput=False)
    Wo16 = nc.declare_dram_parameter("Wo16", [128, KK, D], BF16, isOutput=False)
    Mdiag = nc.declare_dram_parameter("Mdiag", [128, len(PE_TAPS), 128], BF16,
                                      isOutput=False)
    Mcoup = nc.declare_dram_parameter("Mcoup", [128, 2, 2, 2, 128], BF16,
                                      isOutput=False)
    ident_in = nc.declare_dram_parameter("ident_in", [128, 128], BF16,
                                         isOutput=False)
    bo8_in = nc.declare_dram_parameter("bo8_in", [128, MYH], F16, isOutput=False)
    on8_in = nc.declare_dram_parameter("on8_in", [MYH, 128], BF16, isOutput=False)
    wtap_ew = nc.declare_dram_parameter("wtap_ew", [128, len(SC_TAPS)], F32,
                                        isOutput=False)
    Wdv = nc.declare_dram_parameter("Wdv", [128, len(DV_TAPS), 1024], BF16,
                                    isOutput=False)
    ones1 = nc.declare_dram_parameter("ones1", [1, 128], mybir.dt.float32r,
                                      isOutput=False)
    bout1 = nc.declare_dram_parameter("bout1", [1, D], mybir.dt.float32r,
                                      isOutput=False)
    swt = nc.declare_dram_parameter("swt", [128, 2], F32, isOutput=False)
    bqkT = nc.declare_dram_parameter("bqkT", [128, KC], F32, isOutput=False)
    bvT = nc.declare_dram_parameter("bvT", [128, KC], F32, isOutput=False)
    bgT = nc.declare_dram_parameter("bgT", [128, KK], F32, isOutput=False)
    out = nc.declare_dram_parameter("out", [SEQ, D], F32, isOutput=True)

    dbg = {}
    if debug_outputs:
        for name, shape in (("dbg_f0", [128, KC, N]),
                            ("dbg_conv", [128, KC, N]),
                            ("dbg_field", [128, KC, N]),
                            ("dbg_km", [MYH, N]),
                            ("dbg_gate", [128, KK, SEQ]),
                            ("dbg_pgs", [128, KK, SEQ])):
            dbg[name] = nc.declare_dram_parameter(name, shape, BF16,
                                                  isOutput=True)

    # ---- internal DRAM for the exchange (one pairwise AllGather/chunk) ----
    ag_in = [nc.dram_tensor(f"ag_in{c}", [128, N], BF16) for c in range(KC)]
    ag_out = [nc.dram_tensor(f"ag_out{c}", [2, 128, N], BF16)
              for c in range(KC)]

    with tile.TileContext(nc) as tc:
        with (
            tc.tile_pool(name="const", bufs=1) as constp,
            tc.tile_pool(name="p_long", bufs=1) as p_long,
        ):
            # ---- always-resident constants ----
            md_t = constp.tile([128, len(PE_TAPS), 128], BF16)
            nc.sync.dma_start(md_t[:], Mdiag[:])
            mc_t = constp.tile([128, 2, 2, 2, 128], BF16)
            nc.sync.dma_start(mc_t[:], Mcoup[:])
            id_t = constp.tile([128, 128], BF16)
            nc.sync.dma_start(id_t[:], ident_in[:])
            bo_t = constp.tile([128, MYH], F16)
            nc.sync.dma_start(bo_t[:], bo8_in[:])
            on_t = constp.tile([MYH, 128], BF16)
            nc.sync.dma_start(on_t[:], on8_in[:])
            wew_t = constp.tile([128, len(SC_TAPS)], F32)
            nc.sync.dma_start(wew_t[:], wtap_ew[:])
            wdv_t = constp.tile([128, len(DV_TAPS), 1024], BF16)
            nc.sync.dma_start(wdv_t[:], Wdv[:])
            on1_t = constp.tile([1, 128], mybir.dt.float32r)
            nc.sync.dma_start(on1_t[:], ones1[:])
            bo1_t = constp.tile([1, D], mybir.dt.float32r)
            nc.sync.dma_start(bo1_t[:], bout1[:])
            swt_t = constp.tile([128, 2], F32)
            nc.sync.dma_start(swt_t[:], swt[:])
            bqk_t = constp.tile([128, KC], F32)
            nc.sync.dma_start(bqk_t[:], bqkT[:])
            bv_t = constp.tile([128, KC], F32)
            nc.sync.dma_start(bv_t[:], bvT[:])
            bg_t = constp.tile([128, KK], F32)
            nc.sync.dma_start(bg_t[:], bgT[:])

            km_sb = p_long.tile([MYH, N], BF16, tag="km_sb")
            kmb = p_long.tile([128, N], BF16, tag="kmb")

            with tc.tile_pool(name="px16", bufs=1) as px16:
                # allocate first, DMA after the K-phase inputs are queued
                x16 = px16.tile([128, KK, N], F16, tag="x16")
                wv_t = px16.tile([128, KK, KC, 128], F16, tag="wv_t")

                # ================= K phase: kmag =========================
                with (
                    tc.tile_pool(name="px8", bufs=1) as px8,
                    tc.tile_pool(name="pwk", bufs=2) as pwk,
                ):
                    x8 = px8.tile([128, KK, N], F8, tag="x8")
                    wk_t = px8.tile([128, KK, KC, 128], F8, tag="wk_t")
                    nc.sync.dma_start(wk_t[:], Wk8[:])
                    for half in range(2):
                        h0 = half * SEQ
                        for jp in range(KK // 2):
                            nc.sync.dma_start(
                                x8[:, 2 * jp:2 * jp + 2, h0:h0 + SEQ],
                                x8T[:, 2 * jp:2 * jp + 2, h0:h0 + SEQ])
                    # now the big v-phase loads (overlap with K compute)
                    nc.sync.dma_start(x16[:], x16T[:])
                    nc.sync.dma_start(wv_t[:], Wv16[:])

                    with (
                        tc.tile_pool(name="psk", bufs=1, space="PSUM") as psk,
                        tc.tile_pool(name="pskm", bufs=1, space="PSUM") as pskm,
                    ):
                        for half in range(2):
                            h0 = half * SEQ
                            kms = [pskm.tile([MYH, 512], F32, name=f"km{i}",
                                             tag=f"km{i}") for i in range(4)]
                            for c in range(KC):
                                kps = [psk.tile([128, 512], F32, name=f"kp{i}",
                                                tag=f"kp{i}") for i in range(4)]
                                for jp in range(KK // 2):
                                    for bk in range(4):
                                        nc.tensor.matmul(
                                            kps[bk][:],
                                            wk_t[:, 2 * jp:2 * jp + 2, c, :],
                                            x8[:, 2 * jp:2 * jp + 2,
                                               h0 + bk * 512:h0 + bk * 512 + 512],
                                            start=(jp == 0), stop=(jp == 3),
                                            perf_mode=DR)
                                k2t = pwk.tile([128, SEQ], F16, tag="k2t")
                                for bk in range(4):
                                    nc.scalar.activation(
                                        k2t[:, bk * 512:(bk + 1) * 512],
                                        kps[bk][:],
                                        AF.Square, bias=bqk_t[:, c:c + 1])
                                for bk in range(4):
                                    nc.tensor.matmul(
                                        kms[bk][:], bo_t[:],
                                        k2t[:, bk * 512:(bk + 1) * 512],
                                        start=(c == 0), stop=(c == KC - 1))
                            for bk in range(4):
                                nc.scalar.activation(
                                    km_sb[:, h0 + bk * 512:h0 + bk * 512 + 512],
                                    kms[bk][:], AF.Sqrt)

                    # kmag broadcast to all 128 partitions (same all chunks)
                    with tc.tile_pool(name="pskb", bufs=1,
                                      space="PSUM") as pskb:
                        kbs = [pskb.tile([128, 512], F32, name=f"kb{i}",
                                         tag=f"kb{i}") for i in range(4)]
                        for half in range(2):
                            h0 = half * SEQ
                            for bk in range(4):
                                nc.tensor.matmul(
                                    kbs[bk][:], on_t[:],
                                    km_sb[:, h0 + bk * 512:h0 + bk * 512 + 512],
                                    start=True, stop=True)
                            for bk in range(4):
                                nc.scalar.activation(
                                    kmb[:, h0 + bk * 512:h0 + bk * 512 + 512],
                                    kbs[bk][:], AF.Identity)
                if debug_outputs:
                    nc.sync.dma_start(dbg["dbg_km"][:], km_sb[:])

                # ============ V + conv pipeline ==========================
                with (
                    tc.tile_pool(name="pf0", bufs=2) as pf0,
                    tc.tile_pool(name="pwv", bufs=2) as pwv,
                    tc.tile_pool(name="pat", bufs=3) as pat,
                    tc.tile_pool(name="pacc", bufs=2) as pacc,
                    tc.tile_pool(name="pcb", bufs=2) as pcb,
                    tc.tile_pool(name="pcs", bufs=2) as pcs,
                    tc.tile_pool(name="pfl", bufs=1) as pfl,
                    tc.tile_pool(name="psv", bufs=1, space="PSUM") as psv,
                    tc.tile_pool(name="psc", bufs=1, space="PSUM") as psc,
                ):
                    f0ts = {}
                    cbts = {}

                    def v_work(c):
                        f0t = pf0.tile([128, PAD + N], BF16, tag="f0t")
                        f0ts[c] = f0t
                        if c < 2:
                            nc.gpsimd.memset(f0t[:, 0:PAD], 0.0)
                        for half in range(2):
                            h0 = half * SEQ
                            vps = [psv.tile([128, 512], F32, name=f"vp{i}",
                                            tag=f"vp{i}") for i in range(4)]
                            for kk in range(KK):
                                for bk in range(4):
                                    nc.tensor.matmul(
                                        vps[bk][:], wv_t[:, kk, c, :],
                                        x16[:, kk,
                                            h0 + bk * 512:h0 + bk * 512 + 512],
                                        start=(kk == 0), stop=(kk == KK - 1))
                            vbt = pwv.tile([128, SEQ], BF16, tag="vbt")
                            for bk in range(4):
                                nc.scalar.activation(
                                    vbt[:, bk * 512:(bk + 1) * 512], vps[bk][:],
                                    AF.Identity, bias=bv_t[:, c:c + 1])
                            nc.vector.tensor_mul(
                                f0t[:, PAD + h0:PAD + h0 + SEQ], vbt[:],
                                kmb[:, h0:h0 + SEQ])
                        if debug_outputs:
                            nc.sync.dma_start(dbg["dbg_f0"][:, c, :],
                                              f0t[:, PAD:])

                    def conv_ew(c):
                        """small-shift taps: DVE TT-mul (broadcast weights)
                        and ScalarE scale-copy, then DVE/GpSimd add chain."""
                        f0t = f0ts[c]
                        cbt = pcb.tile([128, N], BF16, tag="cbt")
                        cbts[c] = cbt
                        n_terms = len(DV_TAPS) + len(SC_TAPS)
                        for half in range(2):
                            base = PAD + half * SEQ
                            acc = None
                            nadd = 0
                            for i in range(n_terms):
                                att = pat.tile([128, SEQ], BF16, tag="att")
                                if i < len(DV_TAPS):
                                    s = DV_TAPS[i]
                                    for hb in range(2):
                                        o = base - s + hb * 1024
                                        nc.vector.tensor_mul(
                                            att[:, hb * 1024:(hb + 1) * 1024],
                                            f0t[:, o:o + 1024],
                                            wdv_t[:, i, :])
                                else:
                                    j = i - len(DV_TAPS)
                                    s = SC_TAPS[j]
                                    nc.scalar.activation(
                                        att[:], f0t[:, base - s:base - s + SEQ],
                                        AF.Identity, scale=wew_t[:, j:j + 1])
                                if acc is None:
                                    acc = att[:]
                                    continue
                                nadd += 1
                                if i == n_terms - 1:
                                    nxt = cbt[:, half * SEQ:(half + 1) * SEQ]
                                else:
                                    acct = pacc.tile([128, SEQ], BF16,
                                                     tag="acc", name="acct")
                                    nxt = acct[:]
                                if nadd == 3:
                                    nc.gpsimd.tensor_add(nxt, acc, att[:])
                                else:
                                    nc.vector.tensor_add(nxt, acc, att[:])
                                acc = nxt

                    def conv_pe(c):
                        """PE diag taps + chain merge + evict + skip + a2a."""
                        f0t = f0ts.pop(c)
                        cbt = cbts.pop(c)
                        cst = pcs.tile([128, N], BF16, tag="cst")
                        for chalf in range(2):
                            cps = [psc.tile([128, 512], F32, name=f"cp{i}",
                                            tag=f"cp{i}") for i in range(4)]
                            for ti, s in enumerate(PE_TAPS):
                                for bk in range(4):
                                    bg = chalf * 4 + bk
                                    lo = max(0, s - PAD - 512 * bg)
                                    if lo >= 512:
                                        continue
                                    src0 = PAD + 512 * bg + lo - s
                                    nc.tensor.matmul(
                                        cps[bk][:, lo:512], md_t[:, ti, :],
                                        f0t[:, src0:src0 + 512 - lo],
                                        start=(ti == 0), stop=False)
                            for bk in range(4):
                                bg = chalf * 4 + bk
                                nc.tensor.matmul(
                                    cps[bk][:], id_t[:],
                                    cbt[:, bg * 512:(bg + 1) * 512],
                                    start=False, stop=True)
                            for bk in range(4):
                                bg = chalf * 4 + bk
                                nc.scalar.activation(
                                    cst[:, bg * 512:(bg + 1) * 512],
                                    cps[bk][:], AF.Identity)
                        if debug_outputs:
                            nc.sync.dma_start(dbg["dbg_conv"][:, c, :], cst[:])
                        # skip taps: field = conv + sw0*conv[-512] + sw1*conv[-1024]
                        # temporaries borrow the pat/pacc rotating buffers
                        flt = pfl.tile([128, N], BF16, tag="flt")
                        t0a = pat.tile([128, SEQ], BF16, tag="att", name="t0a")
                        t1a = pat.tile([128, SEQ], BF16, tag="att", name="t1a")
                        nc.scalar.activation(t0a[:], cst[:, 0:SEQ],
                                             AF.Identity, scale=swt_t[:, 0:1])
                        nc.scalar.activation(t1a[:], cst[:, 0:SEQ],
                                             AF.Identity, scale=swt_t[:, 1:2])
                        ua = pacc.tile([128, 1024], BF16, tag="ua")
                        nc.vector.tensor_copy(flt[:, 0:512], cst[:, 0:512])
                        nc.vector.tensor_add(flt[:, 512:1024],
                                             cst[:, 512:1024], t0a[:, 0:512])
                        nc.vector.tensor_add(ua[:], t0a[:, 512:1536],
                                             t1a[:, 0:1024])
                        nc.vector.tensor_add(flt[:, 1024:2048],
                                             cst[:, 1024:2048], ua[:])
                        t0b = pat.tile([128, SEQ], BF16, tag="att", name="t0b")
                        t1b = pat.tile([128, SEQ], BF16, tag="att", name="t1b")
                        nc.scalar.activation(t0b[:], cst[:, 1536:1536 + SEQ],
                                             AF.Identity, scale=swt_t[:, 0:1])
                        nc.scalar.activation(t1b[:], cst[:, 1024:1024 + SEQ],
                                             AF.Identity, scale=swt_t[:, 1:2])
                        ub = pacc.tile([128, SEQ], BF16, tag="acc", name="ub")
                        nc.vector.tensor_add(ub[:], t0b[:], t1b[:])
                        nc.vector.tensor_add(flt[:, SEQ:], cst[:, SEQ:],
                                             ub[:])
                        if debug_outputs:
                            nc.sync.dma_start(dbg["dbg_field"][:, c, :], flt[:])
                        nc.sync.dma_start(ag_in[c][:], flt[:])
                        nc.gpsimd.collective_compute(
                            "AllGather", mybir.AluOpType.bypass,
                            replica_groups=GROUPS,
                            ins=[ag_in[c][:]], outs=[ag_out[c][:]])

                    for c in range(KC + 1):
                        if c < KC:
                            v_work(c)
                            conv_ew(c)
                        if c >= 1:
                            conv_pe(c - 1)

            # ================= tail: gate, couple, out ===================
            with (
                tc.tile_pool(name="ptail", bufs=1) as ptail,
                tc.tile_pool(name="pgt", bufs=2) as pgt,
                tc.tile_pool(name="pff", bufs=2) as pff,
                tc.tile_pool(name="pob", bufs=2) as pob,
            ):
                xg = ptail.tile([128, KK, SEQ], F8, tag="xg")
                nc.sync.dma_start(xg[:], xg8[:])
                wg_t = ptail.tile([128, KK, KK, 128], F8, tag="wg_t")
                nc.sync.dma_start(wg_t[:], Wg8[:])
                wo_t = ptail.tile([128, KK, D], BF16, tag="wo_t")
                nc.sync.dma_start(wo_t[:], Wo16[:])
                gate = ptail.tile([128, KK, SEQ], BF16, tag="gate")
                pgs = ptail.tile([128, KK, SEQ], BF16, tag="pgs")

                with tc.tile_pool(name="pscp", bufs=1, space="PSUM") as pscp:
                    with tc.tile_pool(name="psg", bufs=1, space="PSUM") as psg:
                        # gate = sigmoid(x @ (Wq@Wgate) + b'), fp8 DoubleRow
                        for q in range(KK):
                            gps = [psg.tile([128, 512], F32, name=f"gp{i}",
                                            tag=f"gp{i}") for i in range(4)]
                            for jp in range(KK // 2):
                                for bk in range(4):
                                    nc.tensor.matmul(
                                        gps[bk][:],
                                        wg_t[:, 2 * jp:2 * jp + 2, q, :],
                                        xg[:, 2 * jp:2 * jp + 2,
                                           bk * 512:bk * 512 + 512],
                                        start=(jp == 0), stop=(jp == 3),
                                        perf_mode=DR)
                            for bk in range(4):
                                nc.scalar.activation(
                                    gate[:, q, bk * 512:(bk + 1) * 512],
                                    gps[bk][:],
                                    AF.Sigmoid, bias=bg_t[:, q:q + 1])
                        if debug_outputs:
                            nc.sync.dma_start(dbg["dbg_gate"][:], gate[:])

                        # coupling + gate multiply, per exchange chunk.
                        # mc_t[:, jo, hh, ji, :] is host-zeroed unless hh
                        # equals this core's parity, selecting which gather
                        # half (L/R) feeds the couple without runtime offsets.
                        for c in range(KC):
                            fft = pff.tile([128, 2, N], BF16, tag="fft")
                            nc.sync.dma_start(
                                fft[:],
                                ag_out[c][:].rearrange("j p n -> p j n"))
                            for jo in range(2):
                                q = jo * KC + c
                                cpp = [pscp.tile([128, 512], F32,
                                                 name=f"cc{i}", tag=f"cc{i}")
                                       for i in range(4)]
                                for bk in range(4):
                                    first = True
                                    for hh in range(2):
                                        for ji in range(2):
                                            nc.tensor.matmul(
                                                cpp[bk][:],
                                                mc_t[:, jo, hh, ji, :],
                                                fft[:, ji, hh * SEQ + bk * 512:
                                                    hh * SEQ + (bk + 1) * 512],
                                                start=first,
                                                stop=(hh == 1 and ji == 1))
                                            first = False
                                for bk in range(4):
                                    nc.vector.tensor_mul(
                                        pgs[:, q, bk * 512:(bk + 1) * 512],
                                        cpp[bk][:],
                                        gate[:, q, bk * 512:(bk + 1) * 512])
                        if debug_outputs:
                            nc.sync.dma_start(dbg["dbg_pgs"][:], pgs[:])

                    # out = pgs.T @ Wout + bout (bias via rank-1 f32r matmul,
                    # result DMAed PSUM -> DRAM directly)
                    with tc.tile_pool(name="pso", bufs=2, space="PSUM") as pso:
                        for st in range(SEQ // 128):
                            ops = [pso.tile([128, 512], F32, name=f"op{i}",
                                            tag=f"op{i}") for i in range(2)]
                            for cb in range(2):
                                nc.tensor.matmul(
                                    ops[cb][:], on1_t[:],
                                    bo1_t[:, cb * 512:(cb + 1) * 512],
                                    start=True, stop=False)
                            qorder = [0, 1, 2, 4, 5, 6, 3, 7]
                            for qi, q in enumerate(qorder):
                                for cb in range(2):
                                    nc.tensor.matmul(
                                        ops[cb][:],
                                        pgs[:, q, st * 128:(st + 1) * 128],
                                        wo_t[:, q, cb * 512:(cb + 1) * 512],
                                        start=False, stop=(qi == KK - 1))
                            outb = pob.tile([128, D], F32, tag="outb")
                            for cb in range(2):
                                nc.vector.tensor_copy(
                                    outb[:, cb * 512:(cb + 1) * 512],
                                    ops[cb][:])
                            nc.sync.dma_start(out[st * 128:(st + 1) * 128, :],
                                              outb[:])

    nc.compile()
    _PROGRAM_CACHE[key] = nc
    return nc


def _softmax(a, axis):
    a = a - a.max(axis=axis, keepdims=True)
    e = np.exp(a)
    return e / e.sum(axis=axis, keepdims=True)


def _locmap(g):
    """local position p (0..511) -> original channel index (0..1023)."""
    p = np.arange(CH)
    return (MYH * g + p % MYH) * HD + 16 * (p // 128) + (p % 128) // MYH


def _host_prep(inputs):
    x = np.asarray(inputs["x"], np.float32)
    Wqkv = np.asarray(inputs["Wqkv"], np.float64)
    bqkv = np.asarray(inputs["bqkv"], np.float64)
    Wout = np.asarray(inputs["Wout"], np.float64)
    bout = np.asarray(inputs["bout"], np.float32)
    Wgate = np.asarray(inputs["Wgate"], np.float64)
    bgate = np.asarray(inputs["bgate"], np.float64)
    scale_gain = np.asarray(inputs["scale_gain"], np.float64)
    skip_w = np.asarray(inputs["skip_w"], np.float64)
    coupling = np.asarray(inputs["coupling"], np.float64)

    gains = _softmax(scale_gain, axis=0)              # [11, H]
    sw = 1.0 / (1.0 + np.exp(-skip_w))                # [2]
    coup = _softmax(coupling, axis=-1)                # [H, H]

    sidx = {s: i for i, s in enumerate(SHIFTS)}
    wtab = np.zeros((len(SHIFTS), H), np.float64)
    for j in range(N_SCALES):
        d = 1 << j
        for t in range(4):
            wtab[sidx[(3 - t) * d]] += D4[t] * gains[j]

    Wq = Wqkv[:, :D]
    Wk = Wqkv[:, D:2 * D]
    Wv = Wqkv[:, 2 * D:]
    Wqg = Wq @ Wgate                                  # folded gate proj
    bg_full = bqkv[:D] @ Wgate + bgate

    postmap = np.concatenate([_locmap(0), _locmap(1)])  # [1024]

    # uniform (parity-independent) tensors
    r = np.arange(128)
    Wg8 = np.zeros((128, KK, KK, 128), NP_F8)
    Wo16 = np.zeros((128, KK, D), NP_BF)
    bgT = np.zeros((128, KK), np.float32)
    for q in range(KK):
        cols = postmap[q * 128 + np.arange(128)]
        for kk in range(KK):
            Wg8[:, kk, q, :] = Wqg[kk * 128:(kk + 1) * 128, cols].astype(NP_F8)
        Wo16[:, q, :] = Wout[cols, :].astype(NP_BF)
        bgT[:, q] = bg_full[cols].astype(np.float32)

    rr = np.arange(128)[:, None]
    mm = np.arange(128)[None, :]
    same_idx = (rr // MYH) == (mm // MYH)
    coup_blk = np.zeros((2, 2, 128, 128), np.float64)
    for jo in range(2):
        for ji in range(2):
            coup_blk[jo, ji] = (coup[MYH * jo + mm % MYH, MYH * ji + rr % MYH]
                                * same_idx)

    bo8 = np.zeros((128, MYH), np.float16)
    bo8[r, r % MYH] = 1.0
    on8 = np.zeros((MYH, 128), NP_BF)
    on8[r % MYH, r] = 1.0
    ident = np.eye(128, dtype=NP_BF)
    swt = np.broadcast_to(sw.astype(np.float32), (128, 2)).copy()
    ones1 = np.ones((1, 128), np.float32)
    bout1 = bout.reshape(1, D).astype(np.float32)

    shared = dict(Wg8=Wg8, Wo16=Wo16, bgT=bgT, bo8_in=bo8,
                  on8_in=on8, ident_in=ident, swt=swt, ones1=ones1,
                  bout1=bout1)

    in_maps = []
    for core in range(NCORES):
        b, g = core // 2, core % 2
        loc = _locmap(g)

        # coupling stationary: gather-half hh contributes only when hh == g
        Mc = np.zeros((128, 2, 2, 2, 128), NP_BF)
        for jo in range(2):
            for ji in range(2):
                Mc[:, jo, g, ji, :] = coup_blk[jo, ji].astype(NP_BF)
        heads = MYH * g + np.arange(128) % MYH        # head of partition r

        Wk8 = np.zeros((128, KK, KC, 128), NP_F8)
        Wv16 = np.zeros((128, KK, KC, 128), np.float16)
        for c in range(KC):
            cols = loc[c * 128 + np.arange(128)]
            for kk in range(KK):
                Wk8[:, kk, c, :] = Wk[kk * 128:(kk + 1) * 128, cols].astype(NP_F8)
                Wv16[:, kk, c, :] = Wv[kk * 128:(kk + 1) * 128, cols].astype(np.float16)
        bqkT = bqkv[D + loc].reshape(KC, 128).T.copy().astype(np.float32)
        bvT = bqkv[2 * D + loc].reshape(KC, 128).T.copy().astype(np.float32)

        Md = np.zeros((128, len(PE_TAPS), 128), NP_BF)
        for ti, s in enumerate(PE_TAPS):
            Md[r, ti, r] = wtab[sidx[s], heads].astype(NP_BF)
        wew = np.zeros((128, len(SC_TAPS)), np.float32)
        for i, s in enumerate(SC_TAPS):
            wew[:, i] = wtab[sidx[s], heads].astype(np.float32)
        wdv = np.zeros((128, len(DV_TAPS), 1024), NP_BF)
        for i, s in enumerate(DV_TAPS):
            wdv[:, i, :] = np.broadcast_to(
                wtab[sidx[s], heads].astype(NP_BF)[:, None], (128, 1024))

        xb = x[b]                                     # [N, D]
        x16T = np.ascontiguousarray(
            xb.T.reshape(KK, 128, N).transpose(1, 0, 2)).astype(np.float16)
        x8T = x16T.astype(NP_F8)
        xg = xb[g * SEQ:(g + 1) * SEQ, :]             # [SEQ, D]
        xg8 = np.ascontiguousarray(
            xg.T.reshape(KK, 128, SEQ).transpose(1, 0, 2)).astype(NP_F8)

        in_maps.append(dict(x16T=x16T, x8T=x8T, xg8=xg8, Wk8=Wk8, Wv16=Wv16,
                            Mdiag=Md, wtap_ew=wew, Wdv=wdv, bqkT=bqkT,
                            bvT=bvT, Mcoup=Mc, **shared))
    return in_maps


def run_cores(inputs, debug_outputs=False, trace=False):
    nc = _build_program(debug_outputs=debug_outputs)
    in_maps = _host_prep(inputs)
    res = run_bass_kernel_spmd(nc, in_maps, list(range(NCORES)), trace=trace)
    return res


def kernel(**inputs) -> np.ndarray:
    res = run_cores(inputs)
    out = np.empty((B, N, D), np.float32)
    for c in range(NCORES):
        b, g = c // 2, c % 2
        out[b, g * SEQ:(g + 1) * SEQ, :] = res.results[c]["out"]
    return out
